# revision 25
# baseline (speedup 1.0000x reference)
"""CRF-as-RNN mean-field kernel for Trainium2 (Bass/Tile), 8-core SPMD.

Strategy (v2):
- Shard 2 images x 4 row-strips across 8 cores; 84 rows/core (64 owned +
  halo), 5 mean-field iterations shrink validity, no inter-core comms.
- Partitions = 6 row-groups x 21 channels = 126; free = 14 rows x 256 cols
  (+2 halos: 18 x 260 slots).
- The 5x5 spatial gaussian (sigma=0.1) is a delta => sp == q, folded into
  a second stationary mneg2 = (1+wc) * mneg applied to q via PE.
- Bilateral: 24 shifted products on DVE (fp16 2x), accumulated ON THE PE
  via mneg x t_k matmuls into 7 persistent PSUM z-banks (compat transform
  is linear). No DVE adds, no SBUF acc tile, f32 accumulation.
- Softmax: z-banks already hold logits + pairwise (logits fed as f32
  bitcast to f32r: full PE rate at 512 cols). exp/ln on ACT; lnD
  broadcast back via bneg (f32r) matmul; final exp writes q (fp16) or the
  f32 output tile.
- w-precompute, column-sharded: partitions (group, rgb, quarter) = 72;
  per tap: DVE diff, DVE square, PE rgb-reduce -> [24, 390] PSUM x3,
  ACT exp(-50*d2 + ln(spatial)) -> compact [24, 12*1170] fp16; then 84
  strided-partition DMAs replicate rows to the 21 channel partitions
  (w_all [126, 12*4680] fp16).
"""

import math
import sys
from contextlib import ExitStack

import numpy as np

sys.path.insert(0, "/opt/trn_rl_repo")

# ---------------- problem constants (hardcoded per contract) ----------------
B, C, H, W = 2, 21, 256, 256
G, RG = 6, 14                  # row groups per strip, rows per group
P = G * C                      # 126 partitions
F = RG * W                     # 3584 free elems per partition
NT, NV = 18, 260               # q/w map slots: rows -2..15, cols -2..257
STARTS = [0, 54, 118, 172]     # strip start rows
OWN = [(0, 64), (10, 74), (10, 74), (20, 84)]  # owned local-row range
NUM_ITERS = 5
NCH, CH = 7, 512               # softmax chunks (512 px = 2 rows)
NQ, XW, SS = 4, 64, 72         # col chunks: 4 x 64 owned px, 72 stored slots
IM_U, IM_V = 22, 77            # img chunk slots: rows -4..17, 77 cols
KT = NT * SS                   # 1296 map elems per (tap, chunk)
WT = NQ * KT                   # 5184 w elems per tap
PRE_P = G * 3 * NQ             # 72 precompute partitions (g, rgb, chunk)
CPQ = 432                      # precompute PSUM chunk (3 x 432 = 1296)

# spatial gaussian (5x5, sigma=5), normalized
_ax = np.arange(5, dtype=np.float64) - 2
_xx, _yy = np.meshgrid(_ax, _ax, indexing="ij")
_g = np.exp(-(_xx**2 + _yy**2) / (2 * 5.0**2))
SW = (_g / _g.sum()).astype(np.float64)
WC = float(SW[2, 2])           # center weight (spatial only; color=1)
# 12 unique taps (positive half-window); opposite taps share weight maps.
# dy=0 taps first: their muls (and mirrors) don't read halo rows, covering
# the intra-iteration halo-refresh DMA latency.
TAPS = [(0, 1), (0, 2), (1, -2), (1, -1), (1, 0), (1, 1), (1, 2),
        (2, -2), (2, -1), (2, 0), (2, 1), (2, 2)]

_BASS_CACHE = {}


def _build_bass():
    import concourse.bass as bass
    import concourse.mybir as mybir
    from concourse import tile

    f32 = mybir.dt.float32
    f32r = mybir.dt.float32r
    f16 = mybir.dt.float16
    AF = mybir.ActivationFunctionType
    OP = mybir.AluOpType

    nc = bass.Bass("TRN2", target_bir_lowering=False, debug=False,
                   enable_asserts=False)

    lg_d = nc.dram_tensor("lg", [P, F], f32, kind="ExternalInput")
    img_d = nc.dram_tensor("img", [PRE_P, IM_U * IM_V], f32,
                           kind="ExternalInput")
    mneg_d = nc.dram_tensor("mneg", [P, P], f16, kind="ExternalInput")
    mneg2_d = nc.dram_tensor("mneg2", [P, P], f16, kind="ExternalInput")
    st4_d = nc.dram_tensor("st4", [G * NQ, NQ * P], f16,
                            kind="ExternalInput")
    iden_d = nc.dram_tensor("iden", [P, P], f16, kind="ExternalInput")
    onesd_d = nc.dram_tensor("onesd", [P, G], f16, kind="ExternalInput")
    bneg_d = nc.dram_tensor("bneg", [G, P], f16, kind="ExternalInput")
    rmask_d = nc.dram_tensor("rmask", [PRE_P, G * NQ], f16,
                             kind="ExternalInput")
    lns_d = nc.dram_tensor("lns", [G * NQ, 12], f32, kind="ExternalInput")
    wbounce_d = nc.dram_tensor("wbounce", [PRE_P // 3, 12 * KT], f16,
                               kind="Internal")
    qout_d = nc.dram_tensor("qout", [P, F], f32, kind="ExternalOutput")

    with tile.TileContext(nc) as tc, ExitStack() as ctx:
        const_pool = ctx.enter_context(tc.tile_pool(name="const", bufs=1))
        main_pool = ctx.enter_context(tc.tile_pool(name="main", bufs=1))

        mneg_t = const_pool.tile([P, P], f16, tag="mneg")
        nc.sync.dma_start(mneg_t[:], mneg_d.ap())
        mneg2_t = const_pool.tile([P, P], f16, tag="mneg2")
        nc.sync.dma_start(mneg2_t[:], mneg2_d.ap())
        st4_t = const_pool.tile([G * NQ, NQ * P], f16, tag="st4")
        nc.sync.dma_start(st4_t[:], st4_d.ap())
        iden_t = const_pool.tile([P, P], f16, tag="iden")
        nc.sync.dma_start(iden_t[:], iden_d.ap())
        onesd_t = const_pool.tile([P, G], f16, tag="onesd")
        nc.sync.dma_start(onesd_t[:], onesd_d.ap())
        bneg_t = const_pool.tile([G, P], f16, tag="bneg")
        nc.sync.dma_start(bneg_t[:], bneg_d.ap())
        rmask_t = const_pool.tile([PRE_P, G * NQ], f16, tag="rmask")
        nc.sync.dma_start(rmask_t[:], rmask_d.ap())
        lns_t = const_pool.tile([G * NQ, 12], f32, tag="lns")
        nc.sync.dma_start(lns_t[:], lns_d.ap())

        lg_t = main_pool.tile([P, F], f32, tag="lg")
        nc.sync.dma_start(lg_t[:], lg_d.ap())
        lg2_t = main_pool.tile([P, F], f16, tag="lg2")
        nc.scalar.copy(lg2_t[:], lg_t[:])

        # Absorber matmuls: pre-observe every stationary's DMA queue with a
        # tiny dummy matmul so real matmuls carry at most 1 extra wait.
        with tc.tile_pool(name="scrp", bufs=1, space="PSUM") as scrp:
            scr = scrp.tile([G, 2], f32, tag="scr")
            nc.tensor.matmul(scr[:1, :], mneg_t[:, 0:1], mneg_t[:, 0:2],
                             start=True, stop=True)
            nc.tensor.matmul(scr[:1, :], mneg2_t[:, 0:1], mneg2_t[:, 0:2],
                             start=True, stop=True)
            nc.tensor.matmul(scr[:1, :], st4_t[:, 0:1], st4_t[:, 0:2],
                             start=True, stop=True)
            nc.tensor.matmul(scr[:1, :], iden_t[:, 0:1], iden_t[:, 0:2],
                             start=True, stop=True)
            nc.tensor.matmul(scr[:, :], onesd_t[:], onesd_t[:, 0:2],
                             start=True, stop=True)
            nc.tensor.matmul(scr[:1, :], bneg_t[:, 0:1], bneg_t[:, 0:2],
                             start=True, stop=True)
            nc.tensor.matmul(scr[:1, :], rmask_t[:, 0:1], rmask_t[:, 0:2],
                             start=True, stop=True)

        q_ta = main_pool.tile([P, NT * NV], f16, tag="qa")
        nc.vector.memset(q_ta[:], 0.0)
        q_tb = main_pool.tile([P, NT * NV], f16, tag="qb")
        nc.vector.memset(q_tb[:], 0.0)
        q3a = q_ta[:].rearrange("p (t v) -> p t v", v=NV)
        q3b = q_tb[:].rearrange("p (t v) -> p t v", v=NV)

        w_all = main_pool.tile([P, 12 * WT], f16, tag="wall")
        e_pool = ctx.enter_context(tc.tile_pool(name="E", bufs=2))
        ln_pool = ctx.enter_context(tc.tile_pool(name="ln", bufs=2))

        # ---------------- w-map precompute (column-sharded) ----------------
        # d-bank is persistent (shared by init + iteration tails).
        d_pool = ctx.enter_context(tc.tile_pool(name="dps", bufs=1,
                                                space="PSUM"))
        DMA_TAPS = range(8, 12)    # replicated via DRAM-bounce DMA reads
        COMP_TAPS = range(0, 8)    # replicated via PE matmul + ACT/DVE copies
        with tc.tile_pool(name="pre", bufs=1) as prep, \
             tc.tile_pool(name="pre2", bufs=2) as prep2, \
             tc.tile_pool(name="psp", bufs=1, space="PSUM") as psp, \
             tc.tile_pool(name="bcp", bufs=2, space="PSUM") as bcp, \
             tc.tile_pool(name="izp", bufs=2, space="PSUM") as izp:
            img_t = prep.tile([PRE_P, IM_U * IM_V], f32, tag="img")
            nc.sync.dma_start(img_t[:], img_d.ap())
            img3 = img_t[:].rearrange("p (u v) -> p u v", v=IM_V)

            def do_tap(ki):
                dy, dx = TAPS[ki]
                diff_t = prep2.tile([PRE_P, KT], f16, tag="diff")
                diff3 = diff_t[:].rearrange("p (t v) -> p t v", v=SS)
                nc.vector.tensor_sub(
                    diff3[:, :, :],
                    img3[:, 2 + dy:2 + dy + NT, 2 + dx:2 + dx + SS],
                    img3[:, 2:2 + NT, 2:2 + SS],
                )
                sq_t = prep2.tile([PRE_P, KT], f16, tag="sq")
                nc.vector.tensor_mul(sq_t[:], diff_t[:], diff_t[:])
                ctap_t = prep2.tile([PRE_P // 3, KT], f16, tag="ctap")
                for cc in range(3):
                    sl = slice(cc * CPQ, (cc + 1) * CPQ)
                    d2_ps = psp.tile([G * NQ, CPQ], f32, tag="d2")
                    nc.tensor.matmul(d2_ps[:], rmask_t[:], sq_t[:, sl],
                                     start=True, stop=True)
                    nc.scalar.activation(ctap_t[:, sl], d2_ps[:], AF.Exp,
                                         scale=-50.0,
                                         bias=lns_t[0:24, ki:ki + 1])
                if ki in DMA_TAPS:
                    nc.sync.dma_start(
                        wbounce_d.ap()[:, ki * KT:(ki + 1) * KT], ctap_t[:])
                else:
                    # PE replication [(qt,g) -> (g,c)] with K=24 masked
                    # stationaries (col-block qt active for rows (qt,*)).
                    for qt in range(NQ):
                        st = st4_t[:, qt * P:(qt + 1) * P]
                        for cc, (o0, o1) in enumerate(
                                ((0, 512), (512, 1024), (1024, KT))):
                            b_ps = bcp.tile([P, 512], f32, tag="bc",
                                            name="b_ps")
                            nc.tensor.matmul(b_ps[:, 0:o1 - o0], st,
                                             ctap_t[:, o0:o1],
                                             start=True, stop=True)
                            wdst = w_all[:, (qt * 12 + ki) * KT + o0:
                                         (qt * 12 + ki) * KT + o1]
                            if (qt + cc) % 2 == 0:
                                nc.scalar.copy(wdst, b_ps[:, 0:o1 - o0])
                            else:
                                nc.vector.tensor_copy(wdst,
                                                      b_ps[:, 0:o1 - o0])

            for ki in list(DMA_TAPS) + list(COMP_TAPS):
                do_tap(ki)
            # DMA-half broadcast via DRAM bounce, grouped (ch, tap-triple)
            # so each start is [6 parts, 4 qt-blocks of 7776 B].
            for ch in range(C):
                for k0 in (8, 10):
                    src_ap = wbounce_d.ap().rearrange(
                        "(q g) (k f) -> g q k f", g=G, f=KT)[
                        :, :, k0:k0 + 2, :]
                    wall_g = w_all[:].rearrange(
                        "(g c) (q k f) -> g c q k f", c=C, k=12, f=KT)
                    nc.sync.dma_start(
                        wall_g[:, ch, :, k0:k0 + 2, :], src_ap)

            # init: q0 = softmax(logits), rotating PSUM banks (z pool is
            # not allocated yet -- front pools own the banks).
            lg3i = lg_t[:].rearrange("p (r x) -> p r x", x=W)
            for c in range(NCH):
                iz = izp.tile([P, CH], f32, tag="iz", name="iz")
                nc.tensor.matmul(iz[:], iden_t[:],
                                 lg2_t[:, c * CH:(c + 1) * CH],
                                 start=True, stop=False,
                                 skip_group_check=True)
                e_t = e_pool.tile([P, CH], f16, tag="E")
                nc.scalar.activation(e_t[:], iz[:], AF.Exp)
                d_ps = d_pool.tile([G, CH], f32, tag="D")
                nc.tensor.matmul(d_ps[:], onesd_t[:], e_t[:],
                                 start=True, stop=True)
                ln_t = ln_pool.tile([G, CH], f16, tag="ln")
                nc.scalar.activation(ln_t[:], d_ps[:], AF.Ln)
                nc.tensor.matmul(iz[:], bneg_t[:], ln_t[:],
                                 start=False, stop=True,
                                 skip_group_check=True)
                nc.scalar.activation(q3a[:, 2 + 2 * c:4 + 2 * c, 2:2 + W],
                                     iz[:], AF.Exp)

        # ---------------- iteration machinery ----------------
        z_pool = ctx.enter_context(tc.tile_pool(name="zps", bufs=1,
                                                space="PSUM"))
        t_pool = ctx.enter_context(tc.tile_pool(name="tprod", bufs=3))

        def z_banks():
            return [z_pool.tile([P, CH], f32, tag=f"z{c}", name=f"z{c}")
                    for c in range(NCH)]

        lg3 = lg_t[:].rearrange("p (r x) -> p r x", x=W)

        def tail(zs, chunks, q3n, last):
            """exp -> D-reduce -> ln -> -lnD broadcast -> exp(q)."""
            for c in chunks:
                e_t = e_pool.tile([P, CH], f16, tag="E")
                nc.scalar.activation(e_t[:], zs[c][:], AF.Exp)
                d_ps = d_pool.tile([G, CH], f32, tag="D")
                nc.tensor.matmul(d_ps[:], onesd_t[:], e_t[:],
                                 start=True, stop=True)
                ln_t = ln_pool.tile([G, CH], f16, tag="ln")
                nc.scalar.activation(ln_t[:], d_ps[:], AF.Ln)
                nc.tensor.matmul(zs[c][:], bneg_t[:], ln_t[:],
                                 start=False, stop=True,
                                 skip_group_check=True)
                if last:
                    nc.scalar.activation(lg3[:, 2 * c:2 * c + 2, 0:W],
                                         zs[c][:], AF.Exp)
                else:
                    nc.scalar.activation(
                        q3n[:, 2 + 2 * c:4 + 2 * c, 2:2 + W], zs[c][:],
                        AF.Exp)

        # two row-halves per iteration: tails of half A hide under half B's
        # DVE/PE work; ping-pong q (read old, write new) makes that legal.
        HALVES = [(0, 8, range(0, 4)), (8, RG, range(4, NCH))]

        for it in range(NUM_ITERS):
            last = it == NUM_ITERS - 1
            q3o, q3n = (q3a, q3b) if it % 2 == 0 else (q3b, q3a)
            # refresh intra-core group halos of the OLD q
            nc.sync.dma_start(q3o[21:126, 0:2, 0:NV], q3o[0:105, 14:16, 0:NV])
            nc.sync.dma_start(q3o[0:105, 16:18, 0:NV], q3o[21:126, 2:4, 0:NV])

            zs = z_banks()
            for (r0, r1, chunks) in HALVES:
                nr = r1 - r0
                for c in chunks:
                    sl = slice(c * CH, (c + 1) * CH)
                    nc.tensor.matmul(zs[c][:], iden_t[:], lg2_t[:, sl],
                                     start=True, stop=False,
                                     skip_group_check=True)
                    nc.tensor.matmul(zs[c][:], mneg2_t[:],
                                     q3o[:, 2 + 2 * c:4 + 2 * c, 2:2 + W],
                                     start=False, stop=False,
                                     skip_group_check=True)
                for ki, (dy, dx) in enumerate(TAPS):
                    w4 = w_all[:].rearrange(
                        "p (q k t v) -> p q k t v", q=NQ, k=12,
                        v=SS)[:, :, ki]
                    for (qdy, qdx, wdy, wdx) in ((dy, dx, 0, 0),
                                                 (-dy, -dx, -dy, -dx)):
                        q_ap = q3o[:, 2 + qdy + r0:2 + qdy + r1,
                                   2 + qdx:2 + qdx + W].rearrange(
                            "p r (q x) -> p r q x", x=XW)
                        w_ap = w4[:, :, 2 + wdy + r0:2 + wdy + r1,
                                  2 + wdx:2 + wdx + XW].rearrange(
                            "p q r x -> p r q x")
                        t_t = t_pool.tile([P, nr * W], f16, tag=f"t{r0}",
                                          name="t_t")
                        t4 = t_t[:].rearrange("p (r q x) -> p r q x",
                                              q=NQ, x=XW)
                        nc.vector.tensor_mul(t4[:, :, :, :], q_ap, w_ap)
                        for c in chunks:
                            sl = slice((2 * c - r0) * W, (2 * c + 2 - r0) * W)
                            nc.tensor.matmul(
                                zs[c][:], mneg_t[:], t_t[:, sl],
                                start=False, stop=False,
                                skip_group_check=True)
                tail(zs, chunks, q3n, last=last)
                if last:
                    r2 = slice(r0 * W, r1 * W)
                    nc.sync.dma_start(qout_d.ap()[:, r2], lg_t[:, r2])

    _legalize_matmul_waits(nc, mybir)
    return nc


def _legalize_matmul_waits(nc, mybir, max_waits=2):
    """TRN2 ISA sync-wait structs hold few waits per instruction; codegen
    aborts on more. Move excess waits onto InstNoOps (1 wait each) inserted
    right before on the same engine."""
    cap = {}
    for f in nc.m.functions:
        for blk in f.blocks:
            insts = blk.instructions
            out = []
            changed = False
            for i in insts:
                si = getattr(i, "sync_info", None)
                eng = getattr(i, "engine", None)
                max_waits = cap.get(type(i).__name__, 1)
                if (si is not None and eng is not None
                        and len(si.on_wait) > max_waits):
                    waits = list(si.on_wait)
                    keep, move = [], []
                    for w in waits:
                        if "PE" in w.ant_name and len(keep) < max_waits:
                            keep.append(w)
                        else:
                            move.append(w)
                    while len(keep) < max_waits and move:
                        keep.append(move.pop())
                    nop_cap = cap.get("InstNoOp", 1)
                    while move:
                        grp, move = move[:nop_cap], move[nop_cap:]
                        nop = mybir.InstNoOp(
                            name=nc.get_next_instruction_name(),
                            engine=eng, ins=[], outs=[])
                        nop.sync_info = mybir.SyncInfo(on_wait=grp,
                                                       on_update=[])
                        out.append(nop)
                    i.sync_info = mybir.SyncInfo(
                        on_wait=keep, on_update=list(si.on_update))
                    changed = True
                out.append(i)
            if changed:
                blk.instructions = out


def _prep_shards(logits, img, compat):
    """Host-side shard prep -> list of 8 in_maps."""
    mneg = np.kron(np.eye(G), -compat.T.astype(np.float64)).astype(np.float16)
    mneg2 = ((1.0 + WC) * np.kron(np.eye(G), -compat.T.astype(np.float64))
             ).astype(np.float16)
    iden = np.eye(P, dtype=np.float16)
    st4 = np.zeros((G * NQ, NQ * P), np.float16)
    for qt in range(NQ):
        for g in range(G):
            st4[qt * G + g, qt * P + g * C:qt * P + (g + 1) * C] = 1.0
    onesd = np.kron(np.eye(G), np.ones((C, 1))).astype(np.float16)
    bneg = np.kron(np.eye(G), -np.ones((1, C))).astype(np.float16)
    # rmask [(g,rgb,q), (g,q)]: sums rgb
    rmask = np.zeros((PRE_P, G * NQ), np.float16)
    for g in range(G):
        for rgb in range(3):
            for qt in range(NQ):
                rmask[(g * 3 + rgb) * NQ + qt, qt * G + g] = 1.0
    lns = np.tile(
        np.array([math.log(SW[2 + dy, 2 + dx]) for (dy, dx) in TAPS],
                 np.float32)[None, :], (G * NQ, 1))

    in_maps = []
    for core in range(8):
        b, j = divmod(core, 4)
        s = STARTS[j]
        lg = logits[b, :, s:s + 84, :].reshape(C, G, RG, W)
        lg = np.ascontiguousarray(
            lg.transpose(1, 0, 2, 3).reshape(P, F)).astype(np.float32)
        im = np.zeros((G, 3, NQ, IM_U, IM_V), np.float32)
        for g in range(G):
            rbase = s + g * RG - 4
            u0, u1 = max(0, -rbase), min(IM_U, H - rbase)
            for qt in range(NQ):
                cbase = qt * XW - 4
                v0, v1 = max(0, -cbase), min(IM_V, W - cbase)
                im[g, :, qt, u0:u1, v0:v1] = img[
                    b, :, rbase + u0:rbase + u1, cbase + v0:cbase + v1]
        im = im.reshape(PRE_P, IM_U * IM_V)
        in_maps.append({
            "lg": lg, "img": np.ascontiguousarray(im),
            "mneg": mneg, "mneg2": mneg2, "st4": st4, "iden": iden, "onesd": onesd,
            "bneg": bneg, "rmask": rmask, "lns": lns,
        })
    return in_maps


def kernel(**inputs):
    logits = np.asarray(inputs["logits"], dtype=np.float32)
    img = np.asarray(inputs["img"], dtype=np.float32)
    compat = np.asarray(inputs["compat_mat"], dtype=np.float32)

    from concourse.bass_utils import run_bass_kernel_spmd
    if "nc" not in _BASS_CACHE:
        _BASS_CACHE["nc"] = _build_bass()
    nc = _BASS_CACHE["nc"]

    in_maps = _prep_shards(logits, img, compat)
    res = run_bass_kernel_spmd(nc, in_maps, core_ids=list(range(8)))
    _BASS_CACHE["last_result"] = res

    out = np.zeros((B, C, H, W), np.float32)
    for core in range(8):
        b, j = divmod(core, 4)
        s = STARTS[j]
        lo, hi = OWN[j]
        qc = res.results[core]["qout"].reshape(G, C, RG, W)
        qc = qc.transpose(1, 0, 2, 3).reshape(C, 84, W)
        out[b, :, s + lo:s + hi, :] = qc[:, lo:hi, :]
    return out


# revision 26
# speedup vs baseline: 1.0237x; 1.0237x over previous
"""CRF-as-RNN mean-field kernel for Trainium2 (Bass/Tile), 8-core SPMD.

Strategy (v2):
- Shard 2 images x 4 row-strips across 8 cores; 84 rows/core (64 owned +
  halo), 5 mean-field iterations shrink validity, no inter-core comms.
- Partitions = 6 row-groups x 21 channels = 126; free = 14 rows x 256 cols
  (+2 halos: 18 x 260 slots).
- The 5x5 spatial gaussian (sigma=0.1) is a delta => sp == q, folded into
  a second stationary mneg2 = (1+wc) * mneg applied to q via PE.
- Bilateral: 24 shifted products on DVE (fp16 2x), accumulated ON THE PE
  via mneg x t_k matmuls into 7 persistent PSUM z-banks (compat transform
  is linear). No DVE adds, no SBUF acc tile, f32 accumulation.
- Softmax: z-banks already hold logits + pairwise (logits fed as f32
  bitcast to f32r: full PE rate at 512 cols). exp/ln on ACT; lnD
  broadcast back via bneg (f32r) matmul; final exp writes q (fp16) or the
  f32 output tile.
- w-precompute, column-sharded: partitions (group, rgb, quarter) = 72;
  per tap: DVE diff, DVE square, PE rgb-reduce -> [24, 390] PSUM x3,
  ACT exp(-50*d2 + ln(spatial)) -> compact [24, 12*1170] fp16; then 84
  strided-partition DMAs replicate rows to the 21 channel partitions
  (w_all [126, 12*4680] fp16).
"""

import math
import sys
from contextlib import ExitStack

import numpy as np

sys.path.insert(0, "/opt/trn_rl_repo")

# ---------------- problem constants (hardcoded per contract) ----------------
B, C, H, W = 2, 21, 256, 256
G, RG = 6, 14                  # row groups per strip, rows per group
P = G * C                      # 126 partitions
F = RG * W                     # 3584 free elems per partition
NT, NV = 18, 260               # q/w map slots: rows -2..15, cols -2..257
STARTS = [0, 54, 118, 172]     # strip start rows
OWN = [(0, 64), (10, 74), (10, 74), (20, 84)]  # owned local-row range
NUM_ITERS = 5
NCH, CH = 7, 512               # softmax chunks (512 px = 2 rows)
NQ, XW, SS = 4, 64, 72         # col chunks: 4 x 64 owned px, 72 stored slots
IM_U, IM_V = 22, 77            # img chunk slots: rows -4..17, 77 cols
KT = NT * SS                   # 1296 map elems per (tap, chunk)
WT = NQ * KT                   # 5184 w elems per tap
PRE_P = G * 3 * NQ             # 72 precompute partitions (g, rgb, chunk)
CPQ = 432                      # precompute PSUM chunk (3 x 432 = 1296)

# spatial gaussian (5x5, sigma=5), normalized
_ax = np.arange(5, dtype=np.float64) - 2
_xx, _yy = np.meshgrid(_ax, _ax, indexing="ij")
_g = np.exp(-(_xx**2 + _yy**2) / (2 * 5.0**2))
SW = (_g / _g.sum()).astype(np.float64)
WC = float(SW[2, 2])           # center weight (spatial only; color=1)
# 12 unique taps (positive half-window); opposite taps share weight maps.
# dy=0 taps first: their muls (and mirrors) don't read halo rows, covering
# the intra-iteration halo-refresh DMA latency.
TAPS = [(0, 1), (0, 2), (1, -2), (1, -1), (1, 0), (1, 1), (1, 2),
        (2, -2), (2, -1), (2, 0), (2, 1), (2, 2)]

_BASS_CACHE = {}


def _build_bass():
    import concourse.bass as bass
    import concourse.mybir as mybir
    from concourse import tile

    f32 = mybir.dt.float32
    f32r = mybir.dt.float32r
    f16 = mybir.dt.float16
    AF = mybir.ActivationFunctionType
    OP = mybir.AluOpType

    nc = bass.Bass("TRN2", target_bir_lowering=False, debug=False,
                   enable_asserts=False)

    lg_d = nc.dram_tensor("lg", [P, F], f32, kind="ExternalInput")
    img_d = nc.dram_tensor("img", [PRE_P, IM_U * IM_V], f32,
                           kind="ExternalInput")
    mneg_d = nc.dram_tensor("mneg", [P, P], f16, kind="ExternalInput")
    mneg2_d = nc.dram_tensor("mneg2", [P, P], f16, kind="ExternalInput")
    st4_d = nc.dram_tensor("st4", [G * NQ, NQ * P], f16,
                            kind="ExternalInput")
    iden_d = nc.dram_tensor("iden", [P, P], f16, kind="ExternalInput")
    onesd_d = nc.dram_tensor("onesd", [P, G], f16, kind="ExternalInput")
    bneg_d = nc.dram_tensor("bneg", [G, P], f16, kind="ExternalInput")
    rmask_d = nc.dram_tensor("rmask", [PRE_P, G * NQ], f16,
                             kind="ExternalInput")
    lns_d = nc.dram_tensor("lns", [G * NQ, 12], f32, kind="ExternalInput")
    wbounce_d = nc.dram_tensor("wbounce", [PRE_P // 3, 12 * KT], f16,
                               kind="Internal")
    qout_d = nc.dram_tensor("qout", [P, F], f32, kind="ExternalOutput")

    with tile.TileContext(nc) as tc, ExitStack() as ctx:
        const_pool = ctx.enter_context(tc.tile_pool(name="const", bufs=1))
        main_pool = ctx.enter_context(tc.tile_pool(name="main", bufs=1))

        mneg_t = const_pool.tile([P, P], f16, tag="mneg")
        nc.sync.dma_start(mneg_t[:], mneg_d.ap())
        mneg2_t = const_pool.tile([P, P], f16, tag="mneg2")
        nc.sync.dma_start(mneg2_t[:], mneg2_d.ap())
        st4_t = const_pool.tile([G * NQ, NQ * P], f16, tag="st4")
        nc.sync.dma_start(st4_t[:], st4_d.ap())
        iden_t = const_pool.tile([P, P], f16, tag="iden")
        nc.sync.dma_start(iden_t[:], iden_d.ap())
        onesd_t = const_pool.tile([P, G], f16, tag="onesd")
        nc.sync.dma_start(onesd_t[:], onesd_d.ap())
        bneg_t = const_pool.tile([G, P], f16, tag="bneg")
        nc.sync.dma_start(bneg_t[:], bneg_d.ap())
        rmask_t = const_pool.tile([PRE_P, G * NQ], f16, tag="rmask")
        nc.sync.dma_start(rmask_t[:], rmask_d.ap())
        lns_t = const_pool.tile([G * NQ, 12], f32, tag="lns")
        nc.sync.dma_start(lns_t[:], lns_d.ap())

        lg_t = main_pool.tile([P, F], f32, tag="lg")
        nc.sync.dma_start(lg_t[:], lg_d.ap())
        lg2_t = main_pool.tile([P, F], f16, tag="lg2")
        nc.scalar.copy(lg2_t[:], lg_t[:])

        # Absorber matmuls: pre-observe every stationary's DMA queue with a
        # tiny dummy matmul so real matmuls carry at most 1 extra wait.
        with tc.tile_pool(name="scrp", bufs=1, space="PSUM") as scrp:
            scr = scrp.tile([G, 2], f32, tag="scr")
            nc.tensor.matmul(scr[:1, :], mneg_t[:, 0:1], mneg_t[:, 0:2],
                             start=True, stop=True)
            nc.tensor.matmul(scr[:1, :], mneg2_t[:, 0:1], mneg2_t[:, 0:2],
                             start=True, stop=True)
            nc.tensor.matmul(scr[:1, :], st4_t[:, 0:1], st4_t[:, 0:2],
                             start=True, stop=True)
            nc.tensor.matmul(scr[:1, :], iden_t[:, 0:1], iden_t[:, 0:2],
                             start=True, stop=True)
            nc.tensor.matmul(scr[:, :], onesd_t[:], onesd_t[:, 0:2],
                             start=True, stop=True)
            nc.tensor.matmul(scr[:1, :], bneg_t[:, 0:1], bneg_t[:, 0:2],
                             start=True, stop=True)
            nc.tensor.matmul(scr[:1, :], rmask_t[:, 0:1], rmask_t[:, 0:2],
                             start=True, stop=True)

        q_ta = main_pool.tile([P, NT * NV], f16, tag="qa")
        nc.vector.memset(q_ta[:], 0.0)
        q_tb = main_pool.tile([P, NT * NV], f16, tag="qb")
        nc.vector.memset(q_tb[:], 0.0)
        q3a = q_ta[:].rearrange("p (t v) -> p t v", v=NV)
        q3b = q_tb[:].rearrange("p (t v) -> p t v", v=NV)

        w_all = main_pool.tile([P, 12 * WT], f16, tag="wall")
        e_pool = ctx.enter_context(tc.tile_pool(name="E", bufs=2))
        ln_pool = ctx.enter_context(tc.tile_pool(name="ln", bufs=2))

        # ---------------- w-map precompute (column-sharded) ----------------
        # d-bank is persistent (shared by init + iteration tails).
        d_pool = ctx.enter_context(tc.tile_pool(name="dps", bufs=1,
                                                space="PSUM"))
        DMA_TAPS = range(6, 12)    # replicated via DRAM-bounce DMA reads
        COMP_TAPS = range(0, 6)    # replicated via PE matmul + ACT/DVE copies
        with tc.tile_pool(name="pre", bufs=1) as prep, \
             tc.tile_pool(name="pre2", bufs=2) as prep2, \
             tc.tile_pool(name="psp", bufs=1, space="PSUM") as psp, \
             tc.tile_pool(name="bcp", bufs=2, space="PSUM") as bcp, \
             tc.tile_pool(name="izp", bufs=2, space="PSUM") as izp:
            img_t = prep.tile([PRE_P, IM_U * IM_V], f32, tag="img")
            nc.sync.dma_start(img_t[:], img_d.ap())
            img3 = img_t[:].rearrange("p (u v) -> p u v", v=IM_V)

            def do_tap(ki):
                dy, dx = TAPS[ki]
                diff_t = prep2.tile([PRE_P, KT], f16, tag="diff")
                diff3 = diff_t[:].rearrange("p (t v) -> p t v", v=SS)
                nc.vector.tensor_sub(
                    diff3[:, :, :],
                    img3[:, 2 + dy:2 + dy + NT, 2 + dx:2 + dx + SS],
                    img3[:, 2:2 + NT, 2:2 + SS],
                )
                sq_t = prep2.tile([PRE_P, KT], f16, tag="sq")
                nc.vector.tensor_mul(sq_t[:], diff_t[:], diff_t[:])
                ctap_t = prep2.tile([PRE_P // 3, KT], f16, tag="ctap")
                for cc in range(3):
                    sl = slice(cc * CPQ, (cc + 1) * CPQ)
                    d2_ps = psp.tile([G * NQ, CPQ], f32, tag="d2")
                    nc.tensor.matmul(d2_ps[:], rmask_t[:], sq_t[:, sl],
                                     start=True, stop=True)
                    nc.scalar.activation(ctap_t[:, sl], d2_ps[:], AF.Exp,
                                         scale=-50.0,
                                         bias=lns_t[0:24, ki:ki + 1])
                if ki in DMA_TAPS:
                    nc.sync.dma_start(
                        wbounce_d.ap()[:, ki * KT:(ki + 1) * KT], ctap_t[:])
                else:
                    # PE replication [(qt,g) -> (g,c)] with K=24 masked
                    # stationaries (col-block qt active for rows (qt,*)).
                    for qt in range(NQ):
                        st = st4_t[:, qt * P:(qt + 1) * P]
                        for cc, (o0, o1) in enumerate(
                                ((0, 512), (512, 1024), (1024, KT))):
                            b_ps = bcp.tile([P, 512], f32, tag="bc",
                                            name="b_ps")
                            nc.tensor.matmul(b_ps[:, 0:o1 - o0], st,
                                             ctap_t[:, o0:o1],
                                             start=True, stop=True)
                            wdst = w_all[:, (qt * 12 + ki) * KT + o0:
                                         (qt * 12 + ki) * KT + o1]
                            if (qt + cc) % 2 == 0:
                                nc.scalar.copy(wdst, b_ps[:, 0:o1 - o0])
                            else:
                                nc.vector.tensor_copy(wdst,
                                                      b_ps[:, 0:o1 - o0])

            for ki in list(DMA_TAPS) + list(COMP_TAPS):
                do_tap(ki)
            # DMA-half broadcast via DRAM bounce, grouped (ch, tap-triple)
            # so each start is [6 parts, 4 qt-blocks of 7776 B].
            for ch in range(C):
                for k0 in (6, 9):
                    src_ap = wbounce_d.ap().rearrange(
                        "(q g) (k f) -> g q k f", g=G, f=KT)[
                        :, :, k0:k0 + 3, :]
                    wall_g = w_all[:].rearrange(
                        "(g c) (q k f) -> g c q k f", c=C, k=12, f=KT)
                    nc.sync.dma_start(
                        wall_g[:, ch, :, k0:k0 + 3, :], src_ap)

            # init: q0 = softmax(logits), rotating PSUM banks (z pool is
            # not allocated yet -- front pools own the banks).
            lg3i = lg_t[:].rearrange("p (r x) -> p r x", x=W)
            for c in range(NCH):
                iz = izp.tile([P, CH], f32, tag="iz", name="iz")
                nc.tensor.matmul(iz[:], iden_t[:],
                                 lg2_t[:, c * CH:(c + 1) * CH],
                                 start=True, stop=False,
                                 skip_group_check=True)
                e_t = e_pool.tile([P, CH], f16, tag="E")
                nc.scalar.activation(e_t[:], iz[:], AF.Exp)
                d_ps = d_pool.tile([G, CH], f32, tag="D")
                nc.tensor.matmul(d_ps[:], onesd_t[:], e_t[:],
                                 start=True, stop=True)
                ln_t = ln_pool.tile([G, CH], f16, tag="ln")
                nc.scalar.activation(ln_t[:], d_ps[:], AF.Ln)
                nc.tensor.matmul(iz[:], bneg_t[:], ln_t[:],
                                 start=False, stop=True,
                                 skip_group_check=True)
                nc.scalar.activation(q3a[:, 2 + 2 * c:4 + 2 * c, 2:2 + W],
                                     iz[:], AF.Exp)

        # ---------------- iteration machinery ----------------
        z_pool = ctx.enter_context(tc.tile_pool(name="zps", bufs=1,
                                                space="PSUM"))
        t_pool = ctx.enter_context(tc.tile_pool(name="tprod", bufs=3))

        def z_banks():
            return [z_pool.tile([P, CH], f32, tag=f"z{c}", name=f"z{c}")
                    for c in range(NCH)]

        lg3 = lg_t[:].rearrange("p (r x) -> p r x", x=W)

        def tail(zs, chunks, q3n, last):
            """exp -> D-reduce -> ln -> -lnD broadcast -> exp(q)."""
            for c in chunks:
                e_t = e_pool.tile([P, CH], f16, tag="E")
                nc.scalar.activation(e_t[:], zs[c][:], AF.Exp)
                d_ps = d_pool.tile([G, CH], f32, tag="D")
                nc.tensor.matmul(d_ps[:], onesd_t[:], e_t[:],
                                 start=True, stop=True)
                ln_t = ln_pool.tile([G, CH], f16, tag="ln")
                nc.scalar.activation(ln_t[:], d_ps[:], AF.Ln)
                nc.tensor.matmul(zs[c][:], bneg_t[:], ln_t[:],
                                 start=False, stop=True,
                                 skip_group_check=True)
                if last:
                    nc.scalar.activation(lg3[:, 2 * c:2 * c + 2, 0:W],
                                         zs[c][:], AF.Exp)
                else:
                    nc.scalar.activation(
                        q3n[:, 2 + 2 * c:4 + 2 * c, 2:2 + W], zs[c][:],
                        AF.Exp)

        # two row-halves per iteration: tails of half A hide under half B's
        # DVE/PE work; ping-pong q (read old, write new) makes that legal.
        HALVES = [(0, 8, range(0, 4)), (8, RG, range(4, NCH))]

        for it in range(NUM_ITERS):
            last = it == NUM_ITERS - 1
            q3o, q3n = (q3a, q3b) if it % 2 == 0 else (q3b, q3a)
            # refresh intra-core group halos of the OLD q
            nc.sync.dma_start(q3o[21:126, 0:2, 0:NV], q3o[0:105, 14:16, 0:NV])
            nc.sync.dma_start(q3o[0:105, 16:18, 0:NV], q3o[21:126, 2:4, 0:NV])

            zs = z_banks()
            for (r0, r1, chunks) in HALVES:
                nr = r1 - r0
                for c in chunks:
                    sl = slice(c * CH, (c + 1) * CH)
                    nc.tensor.matmul(zs[c][:], iden_t[:], lg2_t[:, sl],
                                     start=True, stop=False,
                                     skip_group_check=True)
                    nc.tensor.matmul(zs[c][:], mneg2_t[:],
                                     q3o[:, 2 + 2 * c:4 + 2 * c, 2:2 + W],
                                     start=False, stop=False,
                                     skip_group_check=True)
                for ki, (dy, dx) in enumerate(TAPS):
                    w4 = w_all[:].rearrange(
                        "p (q k t v) -> p q k t v", q=NQ, k=12,
                        v=SS)[:, :, ki]
                    for (qdy, qdx, wdy, wdx) in ((dy, dx, 0, 0),
                                                 (-dy, -dx, -dy, -dx)):
                        q_ap = q3o[:, 2 + qdy + r0:2 + qdy + r1,
                                   2 + qdx:2 + qdx + W].rearrange(
                            "p r (q x) -> p r q x", x=XW)
                        w_ap = w4[:, :, 2 + wdy + r0:2 + wdy + r1,
                                  2 + wdx:2 + wdx + XW].rearrange(
                            "p q r x -> p r q x")
                        t_t = t_pool.tile([P, nr * W], f16, tag=f"t{r0}",
                                          name="t_t")
                        t4 = t_t[:].rearrange("p (r q x) -> p r q x",
                                              q=NQ, x=XW)
                        nc.vector.tensor_mul(t4[:, :, :, :], q_ap, w_ap)
                        for c in chunks:
                            sl = slice((2 * c - r0) * W, (2 * c + 2 - r0) * W)
                            nc.tensor.matmul(
                                zs[c][:], mneg_t[:], t_t[:, sl],
                                start=False, stop=False,
                                skip_group_check=True)
                tail(zs, chunks, q3n, last=last)
                if last:
                    r2 = slice(r0 * W, r1 * W)
                    nc.sync.dma_start(qout_d.ap()[:, r2], lg_t[:, r2])

    _legalize_matmul_waits(nc, mybir)
    return nc


def _legalize_matmul_waits(nc, mybir, max_waits=2):
    """TRN2 ISA sync-wait structs hold few waits per instruction; codegen
    aborts on more. Move excess waits onto InstNoOps (1 wait each) inserted
    right before on the same engine."""
    cap = {}
    for f in nc.m.functions:
        for blk in f.blocks:
            insts = blk.instructions
            out = []
            changed = False
            for i in insts:
                si = getattr(i, "sync_info", None)
                eng = getattr(i, "engine", None)
                max_waits = cap.get(type(i).__name__, 1)
                if (si is not None and eng is not None
                        and len(si.on_wait) > max_waits):
                    waits = list(si.on_wait)
                    keep, move = [], []
                    for w in waits:
                        if "PE" in w.ant_name and len(keep) < max_waits:
                            keep.append(w)
                        else:
                            move.append(w)
                    while len(keep) < max_waits and move:
                        keep.append(move.pop())
                    nop_cap = cap.get("InstNoOp", 1)
                    while move:
                        grp, move = move[:nop_cap], move[nop_cap:]
                        nop = mybir.InstNoOp(
                            name=nc.get_next_instruction_name(),
                            engine=eng, ins=[], outs=[])
                        nop.sync_info = mybir.SyncInfo(on_wait=grp,
                                                       on_update=[])
                        out.append(nop)
                    i.sync_info = mybir.SyncInfo(
                        on_wait=keep, on_update=list(si.on_update))
                    changed = True
                out.append(i)
            if changed:
                blk.instructions = out


def _prep_shards(logits, img, compat):
    """Host-side shard prep -> list of 8 in_maps."""
    mneg = np.kron(np.eye(G), -compat.T.astype(np.float64)).astype(np.float16)
    mneg2 = ((1.0 + WC) * np.kron(np.eye(G), -compat.T.astype(np.float64))
             ).astype(np.float16)
    iden = np.eye(P, dtype=np.float16)
    st4 = np.zeros((G * NQ, NQ * P), np.float16)
    for qt in range(NQ):
        for g in range(G):
            st4[qt * G + g, qt * P + g * C:qt * P + (g + 1) * C] = 1.0
    onesd = np.kron(np.eye(G), np.ones((C, 1))).astype(np.float16)
    bneg = np.kron(np.eye(G), -np.ones((1, C))).astype(np.float16)
    # rmask [(g,rgb,q), (g,q)]: sums rgb
    rmask = np.zeros((PRE_P, G * NQ), np.float16)
    for g in range(G):
        for rgb in range(3):
            for qt in range(NQ):
                rmask[(g * 3 + rgb) * NQ + qt, qt * G + g] = 1.0
    lns = np.tile(
        np.array([math.log(SW[2 + dy, 2 + dx]) for (dy, dx) in TAPS],
                 np.float32)[None, :], (G * NQ, 1))

    in_maps = []
    for core in range(8):
        b, j = divmod(core, 4)
        s = STARTS[j]
        lg = logits[b, :, s:s + 84, :].reshape(C, G, RG, W)
        lg = np.ascontiguousarray(
            lg.transpose(1, 0, 2, 3).reshape(P, F)).astype(np.float32)
        im = np.zeros((G, 3, NQ, IM_U, IM_V), np.float32)
        for g in range(G):
            rbase = s + g * RG - 4
            u0, u1 = max(0, -rbase), min(IM_U, H - rbase)
            for qt in range(NQ):
                cbase = qt * XW - 4
                v0, v1 = max(0, -cbase), min(IM_V, W - cbase)
                im[g, :, qt, u0:u1, v0:v1] = img[
                    b, :, rbase + u0:rbase + u1, cbase + v0:cbase + v1]
        im = im.reshape(PRE_P, IM_U * IM_V)
        in_maps.append({
            "lg": lg, "img": np.ascontiguousarray(im),
            "mneg": mneg, "mneg2": mneg2, "st4": st4, "iden": iden, "onesd": onesd,
            "bneg": bneg, "rmask": rmask, "lns": lns,
        })
    return in_maps


def kernel(**inputs):
    logits = np.asarray(inputs["logits"], dtype=np.float32)
    img = np.asarray(inputs["img"], dtype=np.float32)
    compat = np.asarray(inputs["compat_mat"], dtype=np.float32)

    from concourse.bass_utils import run_bass_kernel_spmd
    if "nc" not in _BASS_CACHE:
        _BASS_CACHE["nc"] = _build_bass()
    nc = _BASS_CACHE["nc"]

    in_maps = _prep_shards(logits, img, compat)
    res = run_bass_kernel_spmd(nc, in_maps, core_ids=list(range(8)))
    _BASS_CACHE["last_result"] = res

    out = np.zeros((B, C, H, W), np.float32)
    for core in range(8):
        b, j = divmod(core, 4)
        s = STARTS[j]
        lo, hi = OWN[j]
        qc = res.results[core]["qout"].reshape(G, C, RG, W)
        qc = qc.transpose(1, 0, 2, 3).reshape(C, 84, W)
        out[b, :, s + lo:s + hi, :] = qc[:, lo:hi, :]
    return out


# revision 27
# speedup vs baseline: 1.0297x; 1.0058x over previous
"""CRF-as-RNN mean-field kernel for Trainium2 (Bass/Tile), 8-core SPMD.

Strategy (v2):
- Shard 2 images x 4 row-strips across 8 cores; 84 rows/core (64 owned +
  halo), 5 mean-field iterations shrink validity, no inter-core comms.
- Partitions = 6 row-groups x 21 channels = 126; free = 14 rows x 256 cols
  (+2 halos: 18 x 260 slots).
- The 5x5 spatial gaussian (sigma=0.1) is a delta => sp == q, folded into
  a second stationary mneg2 = (1+wc) * mneg applied to q via PE.
- Bilateral: 24 shifted products on DVE (fp16 2x), accumulated ON THE PE
  via mneg x t_k matmuls into 7 persistent PSUM z-banks (compat transform
  is linear). No DVE adds, no SBUF acc tile, f32 accumulation.
- Softmax: z-banks already hold logits + pairwise (logits fed as f32
  bitcast to f32r: full PE rate at 512 cols). exp/ln on ACT; lnD
  broadcast back via bneg (f32r) matmul; final exp writes q (fp16) or the
  f32 output tile.
- w-precompute, column-sharded: partitions (group, rgb, quarter) = 72;
  per tap: DVE diff, DVE square, PE rgb-reduce -> [24, 390] PSUM x3,
  ACT exp(-50*d2 + ln(spatial)) -> compact [24, 12*1170] fp16; then 84
  strided-partition DMAs replicate rows to the 21 channel partitions
  (w_all [126, 12*4680] fp16).
"""

import math
import sys
from contextlib import ExitStack

import numpy as np

sys.path.insert(0, "/opt/trn_rl_repo")

# ---------------- problem constants (hardcoded per contract) ----------------
B, C, H, W = 2, 21, 256, 256
G, RG = 6, 14                  # row groups per strip, rows per group
P = G * C                      # 126 partitions
F = RG * W                     # 3584 free elems per partition
NT, NV = 18, 260               # q/w map slots: rows -2..15, cols -2..257
STARTS = [0, 54, 118, 172]     # strip start rows
OWN = [(0, 64), (10, 74), (10, 74), (20, 84)]  # owned local-row range
NUM_ITERS = 5
NCH, CH = 7, 512               # softmax chunks (512 px = 2 rows)
NQ, XW, SS = 4, 64, 72         # col chunks: 4 x 64 owned px, 72 stored slots
IM_U, IM_V = 22, 77            # img chunk slots: rows -4..17, 77 cols
KT = NT * SS                   # 1296 map elems per (tap, chunk)
WT = NQ * KT                   # 5184 w elems per tap
PRE_P = G * 3 * NQ             # 72 precompute partitions (g, rgb, chunk)
CPQ = 432                      # precompute PSUM chunk (3 x 432 = 1296)

# spatial gaussian (5x5, sigma=5), normalized
_ax = np.arange(5, dtype=np.float64) - 2
_xx, _yy = np.meshgrid(_ax, _ax, indexing="ij")
_g = np.exp(-(_xx**2 + _yy**2) / (2 * 5.0**2))
SW = (_g / _g.sum()).astype(np.float64)
WC = float(SW[2, 2])           # center weight (spatial only; color=1)
# 12 unique taps (positive half-window); opposite taps share weight maps.
# dy=0 taps first: their muls (and mirrors) don't read halo rows, covering
# the intra-iteration halo-refresh DMA latency.
TAPS = [(0, 1), (0, 2), (1, -2), (1, -1), (1, 0), (1, 1), (1, 2),
        (2, -2), (2, -1), (2, 0), (2, 1), (2, 2)]

_BASS_CACHE = {}


def _build_bass():
    import concourse.bass as bass
    import concourse.mybir as mybir
    from concourse import tile

    f32 = mybir.dt.float32
    f32r = mybir.dt.float32r
    f16 = mybir.dt.float16
    AF = mybir.ActivationFunctionType
    OP = mybir.AluOpType

    nc = bass.Bass("TRN2", target_bir_lowering=False, debug=False,
                   enable_asserts=False)

    lg_d = nc.dram_tensor("lg", [P, F], f32, kind="ExternalInput")
    img_d = nc.dram_tensor("img", [PRE_P, IM_U * IM_V], f32,
                           kind="ExternalInput")
    mneg_d = nc.dram_tensor("mneg", [P, P], f16, kind="ExternalInput")
    mneg2_d = nc.dram_tensor("mneg2", [P, P], f16, kind="ExternalInput")
    st4_d = nc.dram_tensor("st4", [G * NQ, NQ * P], f16,
                            kind="ExternalInput")
    iden_d = nc.dram_tensor("iden", [P, P], f16, kind="ExternalInput")
    onesd_d = nc.dram_tensor("onesd", [P, G], f16, kind="ExternalInput")
    bneg_d = nc.dram_tensor("bneg", [G, P], f16, kind="ExternalInput")
    rmask_d = nc.dram_tensor("rmask", [PRE_P, G * NQ], f16,
                             kind="ExternalInput")
    lns_d = nc.dram_tensor("lns", [G * NQ, 12], f32, kind="ExternalInput")
    wbounce_d = nc.dram_tensor("wbounce", [PRE_P // 3, 12 * KT], f16,
                               kind="Internal")
    qout_d = nc.dram_tensor("qout", [P, F], f32, kind="ExternalOutput")

    with tile.TileContext(nc) as tc, ExitStack() as ctx:
        const_pool = ctx.enter_context(tc.tile_pool(name="const", bufs=1))
        main_pool = ctx.enter_context(tc.tile_pool(name="main", bufs=1))

        mneg_t = const_pool.tile([P, P], f16, tag="mneg")
        nc.sync.dma_start(mneg_t[:], mneg_d.ap())
        mneg2_t = const_pool.tile([P, P], f16, tag="mneg2")
        nc.sync.dma_start(mneg2_t[:], mneg2_d.ap())
        st4_t = const_pool.tile([G * NQ, NQ * P], f16, tag="st4")
        nc.sync.dma_start(st4_t[:], st4_d.ap())
        iden_t = const_pool.tile([P, P], f16, tag="iden")
        nc.sync.dma_start(iden_t[:], iden_d.ap())
        onesd_t = const_pool.tile([P, G], f16, tag="onesd")
        nc.sync.dma_start(onesd_t[:], onesd_d.ap())
        bneg_t = const_pool.tile([G, P], f16, tag="bneg")
        nc.sync.dma_start(bneg_t[:], bneg_d.ap())
        rmask_t = const_pool.tile([PRE_P, G * NQ], f16, tag="rmask")
        nc.sync.dma_start(rmask_t[:], rmask_d.ap())
        lns_t = const_pool.tile([G * NQ, 12], f32, tag="lns")
        nc.sync.dma_start(lns_t[:], lns_d.ap())

        lg_t = main_pool.tile([P, F], f32, tag="lg")
        nc.sync.dma_start(lg_t[:], lg_d.ap())
        lg2_t = main_pool.tile([P, F], f16, tag="lg2")
        nc.scalar.copy(lg2_t[:], lg_t[:])

        # Absorber matmuls: pre-observe every stationary's DMA queue with a
        # tiny dummy matmul so real matmuls carry at most 1 extra wait.
        with tc.tile_pool(name="scrp", bufs=1, space="PSUM") as scrp:
            scr = scrp.tile([G, 2], f32, tag="scr")
            nc.tensor.matmul(scr[:1, :], mneg_t[:, 0:1], mneg_t[:, 0:2],
                             start=True, stop=True)
            nc.tensor.matmul(scr[:1, :], mneg2_t[:, 0:1], mneg2_t[:, 0:2],
                             start=True, stop=True)
            nc.tensor.matmul(scr[:1, :], st4_t[:, 0:1], st4_t[:, 0:2],
                             start=True, stop=True)
            nc.tensor.matmul(scr[:1, :], iden_t[:, 0:1], iden_t[:, 0:2],
                             start=True, stop=True)
            nc.tensor.matmul(scr[:, :], onesd_t[:], onesd_t[:, 0:2],
                             start=True, stop=True)
            nc.tensor.matmul(scr[:1, :], bneg_t[:, 0:1], bneg_t[:, 0:2],
                             start=True, stop=True)
            nc.tensor.matmul(scr[:1, :], rmask_t[:, 0:1], rmask_t[:, 0:2],
                             start=True, stop=True)

        q_ta = main_pool.tile([P, NT * NV], f16, tag="qa")
        nc.vector.memset(q_ta[:], 0.0)
        q_tb = main_pool.tile([P, NT * NV], f16, tag="qb")
        nc.vector.memset(q_tb[:], 0.0)
        q3a = q_ta[:].rearrange("p (t v) -> p t v", v=NV)
        q3b = q_tb[:].rearrange("p (t v) -> p t v", v=NV)

        w_all = main_pool.tile([P, 12 * WT], f16, tag="wall")
        e_pool = ctx.enter_context(tc.tile_pool(name="E", bufs=2))
        ln_pool = ctx.enter_context(tc.tile_pool(name="ln", bufs=2))

        # ---------------- w-map precompute (column-sharded) ----------------
        # d-bank is persistent (shared by init + iteration tails).
        d_pool = ctx.enter_context(tc.tile_pool(name="dps", bufs=1,
                                                space="PSUM"))
        DMA_TAPS = range(6, 12)    # replicated via DRAM-bounce DMA reads
        COMP_TAPS = range(0, 6)    # replicated via PE matmul + ACT/DVE copies
        with tc.tile_pool(name="pre", bufs=1) as prep, \
             tc.tile_pool(name="pre2", bufs=2) as prep2, \
             tc.tile_pool(name="psp", bufs=1, space="PSUM") as psp, \
             tc.tile_pool(name="bcp", bufs=2, space="PSUM") as bcp, \
             tc.tile_pool(name="izp", bufs=2, space="PSUM") as izp:
            img_t = prep.tile([PRE_P, IM_U * IM_V], f32, tag="img")
            nc.sync.dma_start(img_t[:], img_d.ap())
            img3 = img_t[:].rearrange("p (u v) -> p u v", v=IM_V)

            def do_tap(ki):
                dy, dx = TAPS[ki]
                diff_t = prep2.tile([PRE_P, KT], f16, tag="diff")
                diff3 = diff_t[:].rearrange("p (t v) -> p t v", v=SS)
                nc.vector.tensor_sub(
                    diff3[:, :, :],
                    img3[:, 2 + dy:2 + dy + NT, 2 + dx:2 + dx + SS],
                    img3[:, 2:2 + NT, 2:2 + SS],
                )
                sq_t = prep2.tile([PRE_P, KT], f16, tag="sq")
                nc.vector.tensor_mul(sq_t[:], diff_t[:], diff_t[:])
                ctap_t = prep2.tile([PRE_P // 3, KT], f16, tag="ctap")
                for cc in range(3):
                    sl = slice(cc * CPQ, (cc + 1) * CPQ)
                    d2_ps = psp.tile([G * NQ, CPQ], f32, tag="d2")
                    nc.tensor.matmul(d2_ps[:], rmask_t[:], sq_t[:, sl],
                                     start=True, stop=True)
                    nc.scalar.activation(ctap_t[:, sl], d2_ps[:], AF.Exp,
                                         scale=-50.0,
                                         bias=lns_t[0:24, ki:ki + 1])
                if ki in DMA_TAPS:
                    nc.sync.dma_start(
                        wbounce_d.ap()[:, ki * KT:(ki + 1) * KT], ctap_t[:])
                else:
                    # PE replication [(qt,g) -> (g,c)] with K=24 masked
                    # stationaries (col-block qt active for rows (qt,*)).
                    for qt in range(NQ):
                        st = st4_t[:, qt * P:(qt + 1) * P]
                        for cc, (o0, o1) in enumerate(
                                ((0, 512), (512, 1024), (1024, KT))):
                            b_ps = bcp.tile([P, 512], f32, tag="bc",
                                            name="b_ps")
                            nc.tensor.matmul(b_ps[:, 0:o1 - o0], st,
                                             ctap_t[:, o0:o1],
                                             start=True, stop=True)
                            wdst = w_all[:, (qt * 12 + ki) * KT + o0:
                                         (qt * 12 + ki) * KT + o1]
                            if (qt + cc) % 2 == 0:
                                nc.scalar.copy(wdst, b_ps[:, 0:o1 - o0])
                            else:
                                nc.vector.tensor_copy(wdst,
                                                      b_ps[:, 0:o1 - o0])

            for ki in list(DMA_TAPS) + list(COMP_TAPS):
                do_tap(ki)
            # DMA-half broadcast via DRAM bounce, grouped (ch, tap-triple)
            # so each start is [6 parts, 4 qt-blocks of 7776 B].
            for ch in range(C):
                for k0 in (6, 9):
                    src_ap = wbounce_d.ap().rearrange(
                        "(q g) (k f) -> g q k f", g=G, f=KT)[
                        :, :, k0:k0 + 3, :]
                    wall_g = w_all[:].rearrange(
                        "(g c) (q k f) -> g c q k f", c=C, k=12, f=KT)
                    nc.sync.dma_start(
                        wall_g[:, ch, :, k0:k0 + 3, :], src_ap)

            # init: q0 = softmax(logits), rotating PSUM banks (z pool is
            # not allocated yet -- front pools own the banks).
            lg3i = lg_t[:].rearrange("p (r x) -> p r x", x=W)
            for c in range(NCH):
                iz = izp.tile([P, CH], f32, tag="iz", name="iz")
                nc.tensor.matmul(iz[:], iden_t[:],
                                 lg2_t[:, c * CH:(c + 1) * CH],
                                 start=True, stop=False,
                                 skip_group_check=True)
                e_t = e_pool.tile([P, CH], f16, tag="E")
                nc.scalar.activation(e_t[:], iz[:], AF.Exp)
                d_ps = d_pool.tile([G, CH], f32, tag="D")
                nc.tensor.matmul(d_ps[:], onesd_t[:], e_t[:],
                                 start=True, stop=True)
                ln_t = ln_pool.tile([G, CH], f16, tag="ln")
                nc.scalar.activation(ln_t[:], d_ps[:], AF.Ln)
                nc.tensor.matmul(iz[:], bneg_t[:], ln_t[:],
                                 start=False, stop=True,
                                 skip_group_check=True)
                nc.scalar.activation(q3a[:, 2 + 2 * c:4 + 2 * c, 2:2 + W],
                                     iz[:], AF.Exp)
                if c == 0:
                    nc.sync.dma_start(q3a[0:105, 16:18, 0:NV],
                                      q3a[21:126, 2:4, 0:NV])
                elif c == 6:
                    nc.sync.dma_start(q3a[21:126, 0:2, 0:NV],
                                      q3a[0:105, 14:16, 0:NV])

        # ---------------- iteration machinery ----------------
        z_pool = ctx.enter_context(tc.tile_pool(name="zps", bufs=1,
                                                space="PSUM"))
        t_pool = ctx.enter_context(tc.tile_pool(name="tprod", bufs=3))

        def z_banks():
            return [z_pool.tile([P, CH], f32, tag=f"z{c}", name=f"z{c}")
                    for c in range(NCH)]

        lg3 = lg_t[:].rearrange("p (r x) -> p r x", x=W)

        def tail(zs, chunks, q3n, last):
            """exp -> D-reduce -> ln -> -lnD broadcast -> exp(q).
            Halo-refresh DMAs for the just-written q fire right after the
            producing chunk (0 -> up-halo, 6 -> down-halo)."""
            for c in chunks:
                e_t = e_pool.tile([P, CH], f16, tag="E")
                nc.scalar.activation(e_t[:], zs[c][:], AF.Exp)
                d_ps = d_pool.tile([G, CH], f32, tag="D")
                nc.tensor.matmul(d_ps[:], onesd_t[:], e_t[:],
                                 start=True, stop=True)
                ln_t = ln_pool.tile([G, CH], f16, tag="ln")
                nc.scalar.activation(ln_t[:], d_ps[:], AF.Ln)
                nc.tensor.matmul(zs[c][:], bneg_t[:], ln_t[:],
                                 start=False, stop=True,
                                 skip_group_check=True)
                if last:
                    nc.scalar.activation(lg3[:, 2 * c:2 * c + 2, 0:W],
                                         zs[c][:], AF.Exp)
                else:
                    nc.scalar.activation(
                        q3n[:, 2 + 2 * c:4 + 2 * c, 2:2 + W], zs[c][:],
                        AF.Exp)
                    if c == 0:
                        nc.sync.dma_start(q3n[0:105, 16:18, 0:NV],
                                          q3n[21:126, 2:4, 0:NV])
                    elif c == 6:
                        nc.sync.dma_start(q3n[21:126, 0:2, 0:NV],
                                          q3n[0:105, 14:16, 0:NV])

        # two row-halves per iteration: tails of half A hide under half B's
        # DVE/PE work; ping-pong q (read old, write new) makes that legal.
        HALVES = [(0, 8, (0, 1, 2, 3)), (8, RG, (6, 4, 5))]

        for it in range(NUM_ITERS):
            last = it == NUM_ITERS - 1
            q3o, q3n = (q3a, q3b) if it % 2 == 0 else (q3b, q3a)
            zs = z_banks()
            for (r0, r1, chunks) in HALVES:
                nr = r1 - r0
                for c in chunks:
                    sl = slice(c * CH, (c + 1) * CH)
                    nc.tensor.matmul(zs[c][:], iden_t[:], lg2_t[:, sl],
                                     start=True, stop=False,
                                     skip_group_check=True)
                    nc.tensor.matmul(zs[c][:], mneg2_t[:],
                                     q3o[:, 2 + 2 * c:4 + 2 * c, 2:2 + W],
                                     start=False, stop=False,
                                     skip_group_check=True)
                for ki, (dy, dx) in enumerate(TAPS):
                    w4 = w_all[:].rearrange(
                        "p (q k t v) -> p q k t v", q=NQ, k=12,
                        v=SS)[:, :, ki]
                    for (qdy, qdx, wdy, wdx) in ((dy, dx, 0, 0),
                                                 (-dy, -dx, -dy, -dx)):
                        q_ap = q3o[:, 2 + qdy + r0:2 + qdy + r1,
                                   2 + qdx:2 + qdx + W].rearrange(
                            "p r (q x) -> p r q x", x=XW)
                        w_ap = w4[:, :, 2 + wdy + r0:2 + wdy + r1,
                                  2 + wdx:2 + wdx + XW].rearrange(
                            "p q r x -> p r q x")
                        t_t = t_pool.tile([P, nr * W], f16, tag=f"t{r0}",
                                          name="t_t")
                        t4 = t_t[:].rearrange("p (r q x) -> p r q x",
                                              q=NQ, x=XW)
                        nc.vector.tensor_mul(t4[:, :, :, :], q_ap, w_ap)
                        for c in chunks:
                            sl = slice((2 * c - r0) * W, (2 * c + 2 - r0) * W)
                            nc.tensor.matmul(
                                zs[c][:], mneg_t[:], t_t[:, sl],
                                start=False, stop=False,
                                skip_group_check=True)
                tail(zs, chunks, q3n, last=last)
                if last:
                    r2 = slice(r0 * W, r1 * W)
                    nc.sync.dma_start(qout_d.ap()[:, r2], lg_t[:, r2])

    _legalize_matmul_waits(nc, mybir)
    return nc


def _legalize_matmul_waits(nc, mybir, max_waits=2):
    """TRN2 ISA sync-wait structs hold few waits per instruction; codegen
    aborts on more. Move excess waits onto InstNoOps (1 wait each) inserted
    right before on the same engine."""
    cap = {}
    for f in nc.m.functions:
        for blk in f.blocks:
            insts = blk.instructions
            out = []
            changed = False
            for i in insts:
                si = getattr(i, "sync_info", None)
                eng = getattr(i, "engine", None)
                max_waits = cap.get(type(i).__name__, 1)
                if (si is not None and eng is not None
                        and len(si.on_wait) > max_waits):
                    waits = list(si.on_wait)
                    keep, move = [], []
                    for w in waits:
                        if "PE" in w.ant_name and len(keep) < max_waits:
                            keep.append(w)
                        else:
                            move.append(w)
                    while len(keep) < max_waits and move:
                        keep.append(move.pop())
                    nop_cap = cap.get("InstNoOp", 1)
                    while move:
                        grp, move = move[:nop_cap], move[nop_cap:]
                        nop = mybir.InstNoOp(
                            name=nc.get_next_instruction_name(),
                            engine=eng, ins=[], outs=[])
                        nop.sync_info = mybir.SyncInfo(on_wait=grp,
                                                       on_update=[])
                        out.append(nop)
                    i.sync_info = mybir.SyncInfo(
                        on_wait=keep, on_update=list(si.on_update))
                    changed = True
                out.append(i)
            if changed:
                blk.instructions = out


def _prep_shards(logits, img, compat):
    """Host-side shard prep -> list of 8 in_maps."""
    mneg = np.kron(np.eye(G), -compat.T.astype(np.float64)).astype(np.float16)
    mneg2 = ((1.0 + WC) * np.kron(np.eye(G), -compat.T.astype(np.float64))
             ).astype(np.float16)
    iden = np.eye(P, dtype=np.float16)
    st4 = np.zeros((G * NQ, NQ * P), np.float16)
    for qt in range(NQ):
        for g in range(G):
            st4[qt * G + g, qt * P + g * C:qt * P + (g + 1) * C] = 1.0
    onesd = np.kron(np.eye(G), np.ones((C, 1))).astype(np.float16)
    bneg = np.kron(np.eye(G), -np.ones((1, C))).astype(np.float16)
    # rmask [(g,rgb,q), (g,q)]: sums rgb
    rmask = np.zeros((PRE_P, G * NQ), np.float16)
    for g in range(G):
        for rgb in range(3):
            for qt in range(NQ):
                rmask[(g * 3 + rgb) * NQ + qt, qt * G + g] = 1.0
    lns = np.tile(
        np.array([math.log(SW[2 + dy, 2 + dx]) for (dy, dx) in TAPS],
                 np.float32)[None, :], (G * NQ, 1))

    in_maps = []
    for core in range(8):
        b, j = divmod(core, 4)
        s = STARTS[j]
        lg = logits[b, :, s:s + 84, :].reshape(C, G, RG, W)
        lg = np.ascontiguousarray(
            lg.transpose(1, 0, 2, 3).reshape(P, F)).astype(np.float32)
        im = np.zeros((G, 3, NQ, IM_U, IM_V), np.float32)
        for g in range(G):
            rbase = s + g * RG - 4
            u0, u1 = max(0, -rbase), min(IM_U, H - rbase)
            for qt in range(NQ):
                cbase = qt * XW - 4
                v0, v1 = max(0, -cbase), min(IM_V, W - cbase)
                im[g, :, qt, u0:u1, v0:v1] = img[
                    b, :, rbase + u0:rbase + u1, cbase + v0:cbase + v1]
        im = im.reshape(PRE_P, IM_U * IM_V)
        in_maps.append({
            "lg": lg, "img": np.ascontiguousarray(im),
            "mneg": mneg, "mneg2": mneg2, "st4": st4, "iden": iden, "onesd": onesd,
            "bneg": bneg, "rmask": rmask, "lns": lns,
        })
    return in_maps


def kernel(**inputs):
    logits = np.asarray(inputs["logits"], dtype=np.float32)
    img = np.asarray(inputs["img"], dtype=np.float32)
    compat = np.asarray(inputs["compat_mat"], dtype=np.float32)

    from concourse.bass_utils import run_bass_kernel_spmd
    if "nc" not in _BASS_CACHE:
        _BASS_CACHE["nc"] = _build_bass()
    nc = _BASS_CACHE["nc"]

    in_maps = _prep_shards(logits, img, compat)
    res = run_bass_kernel_spmd(nc, in_maps, core_ids=list(range(8)))
    _BASS_CACHE["last_result"] = res

    out = np.zeros((B, C, H, W), np.float32)
    for core in range(8):
        b, j = divmod(core, 4)
        s = STARTS[j]
        lo, hi = OWN[j]
        qc = res.results[core]["qout"].reshape(G, C, RG, W)
        qc = qc.transpose(1, 0, 2, 3).reshape(C, 84, W)
        out[b, :, s + lo:s + hi, :] = qc[:, lo:hi, :]
    return out


# revision 28
# speedup vs baseline: 1.0347x; 1.0049x over previous
"""CRF-as-RNN mean-field kernel for Trainium2 (Bass/Tile), 8-core SPMD.

Strategy (v2):
- Shard 2 images x 4 row-strips across 8 cores; 84 rows/core (64 owned +
  halo), 5 mean-field iterations shrink validity, no inter-core comms.
- Partitions = 6 row-groups x 21 channels = 126; free = 14 rows x 256 cols
  (+2 halos: 18 x 260 slots).
- The 5x5 spatial gaussian (sigma=0.1) is a delta => sp == q, folded into
  a second stationary mneg2 = (1+wc) * mneg applied to q via PE.
- Bilateral: 24 shifted products on DVE (fp16 2x), accumulated ON THE PE
  via mneg x t_k matmuls into 7 persistent PSUM z-banks (compat transform
  is linear). No DVE adds, no SBUF acc tile, f32 accumulation.
- Softmax: z-banks already hold logits + pairwise (logits fed as f32
  bitcast to f32r: full PE rate at 512 cols). exp/ln on ACT; lnD
  broadcast back via bneg (f32r) matmul; final exp writes q (fp16) or the
  f32 output tile.
- w-precompute, column-sharded: partitions (group, rgb, quarter) = 72;
  per tap: DVE diff, DVE square, PE rgb-reduce -> [24, 390] PSUM x3,
  ACT exp(-50*d2 + ln(spatial)) -> compact [24, 12*1170] fp16; then 84
  strided-partition DMAs replicate rows to the 21 channel partitions
  (w_all [126, 12*4680] fp16).
"""

import math
import sys
from contextlib import ExitStack

import numpy as np

sys.path.insert(0, "/opt/trn_rl_repo")

# ---------------- problem constants (hardcoded per contract) ----------------
B, C, H, W = 2, 21, 256, 256
G, RG = 6, 14                  # row groups per strip, rows per group
P = G * C                      # 126 partitions
F = RG * W                     # 3584 free elems per partition
NT, NV = 18, 260               # q/w map slots: rows -2..15, cols -2..257
STARTS = [0, 54, 118, 172]     # strip start rows
OWN = [(0, 64), (10, 74), (10, 74), (20, 84)]  # owned local-row range
NUM_ITERS = 5
NCH, CH = 7, 512               # softmax chunks (512 px = 2 rows)
NQ, XW, SS = 4, 64, 72         # col chunks: 4 x 64 owned px, 72 stored slots
IM_U, IM_V = 22, 77            # img chunk slots: rows -4..17, 77 cols
KT = NT * SS                   # 1296 map elems per (tap, chunk)
WT = NQ * KT                   # 5184 w elems per tap
PRE_P = G * 3 * NQ             # 72 precompute partitions (g, rgb, chunk)
CPQ = 432                      # precompute PSUM chunk (3 x 432 = 1296)

# spatial gaussian (5x5, sigma=5), normalized
_ax = np.arange(5, dtype=np.float64) - 2
_xx, _yy = np.meshgrid(_ax, _ax, indexing="ij")
_g = np.exp(-(_xx**2 + _yy**2) / (2 * 5.0**2))
SW = (_g / _g.sum()).astype(np.float64)
WC = float(SW[2, 2])           # center weight (spatial only; color=1)
# 12 unique taps (positive half-window); opposite taps share weight maps.
# dy=0 taps first: their muls (and mirrors) don't read halo rows, covering
# the intra-iteration halo-refresh DMA latency.
TAPS = [(0, 1), (0, 2), (1, -2), (1, -1), (1, 0), (1, 1), (1, 2),
        (2, -2), (2, -1), (2, 0), (2, 1), (2, 2)]

_BASS_CACHE = {}


def _build_bass():
    import concourse.bass as bass
    import concourse.mybir as mybir
    from concourse import tile

    f32 = mybir.dt.float32
    f32r = mybir.dt.float32r
    f16 = mybir.dt.float16
    AF = mybir.ActivationFunctionType
    OP = mybir.AluOpType

    nc = bass.Bass("TRN2", target_bir_lowering=False, debug=False,
                   enable_asserts=False)

    lg_d = nc.dram_tensor("lg", [P, F], f32, kind="ExternalInput")
    img_d = nc.dram_tensor("img", [PRE_P, IM_U * IM_V], f16,
                           kind="ExternalInput")
    mneg_d = nc.dram_tensor("mneg", [P, P], f16, kind="ExternalInput")
    mneg2_d = nc.dram_tensor("mneg2", [P, P], f16, kind="ExternalInput")
    st4_d = nc.dram_tensor("st4", [G * NQ, NQ * P], f16,
                            kind="ExternalInput")
    iden_d = nc.dram_tensor("iden", [P, P], f16, kind="ExternalInput")
    onesd_d = nc.dram_tensor("onesd", [P, G], f16, kind="ExternalInput")
    bneg_d = nc.dram_tensor("bneg", [G, P], f16, kind="ExternalInput")
    rmask_d = nc.dram_tensor("rmask", [PRE_P, G * NQ], f16,
                             kind="ExternalInput")
    lns_d = nc.dram_tensor("lns", [G * NQ, 12], f32, kind="ExternalInput")
    wbounce_d = nc.dram_tensor("wbounce", [PRE_P // 3, 12 * KT], f16,
                               kind="Internal")
    qout_d = nc.dram_tensor("qout", [P, F], f32, kind="ExternalOutput")

    with tile.TileContext(nc) as tc, ExitStack() as ctx:
        const_pool = ctx.enter_context(tc.tile_pool(name="const", bufs=1))
        main_pool = ctx.enter_context(tc.tile_pool(name="main", bufs=1))

        mneg_t = const_pool.tile([P, P], f16, tag="mneg")
        nc.sync.dma_start(mneg_t[:], mneg_d.ap())
        mneg2_t = const_pool.tile([P, P], f16, tag="mneg2")
        nc.sync.dma_start(mneg2_t[:], mneg2_d.ap())
        st4_t = const_pool.tile([G * NQ, NQ * P], f16, tag="st4")
        nc.sync.dma_start(st4_t[:], st4_d.ap())
        iden_t = const_pool.tile([P, P], f16, tag="iden")
        nc.sync.dma_start(iden_t[:], iden_d.ap())
        onesd_t = const_pool.tile([P, G], f16, tag="onesd")
        nc.sync.dma_start(onesd_t[:], onesd_d.ap())
        bneg_t = const_pool.tile([G, P], f16, tag="bneg")
        nc.sync.dma_start(bneg_t[:], bneg_d.ap())
        rmask_t = const_pool.tile([PRE_P, G * NQ], f16, tag="rmask")
        nc.sync.dma_start(rmask_t[:], rmask_d.ap())
        lns_t = const_pool.tile([G * NQ, 12], f32, tag="lns")
        nc.sync.dma_start(lns_t[:], lns_d.ap())

        lg_t = main_pool.tile([P, F], f32, tag="lg")
        nc.sync.dma_start(lg_t[:], lg_d.ap())
        lg2_t = main_pool.tile([P, F], f16, tag="lg2")
        nc.scalar.copy(lg2_t[:], lg_t[:])

        # Absorber matmuls: pre-observe every stationary's DMA queue with a
        # tiny dummy matmul so real matmuls carry at most 1 extra wait.
        with tc.tile_pool(name="scrp", bufs=1, space="PSUM") as scrp:
            scr = scrp.tile([G, 2], f32, tag="scr")
            nc.tensor.matmul(scr[:1, :], mneg_t[:, 0:1], mneg_t[:, 0:2],
                             start=True, stop=True)
            nc.tensor.matmul(scr[:1, :], mneg2_t[:, 0:1], mneg2_t[:, 0:2],
                             start=True, stop=True)
            nc.tensor.matmul(scr[:1, :], st4_t[:, 0:1], st4_t[:, 0:2],
                             start=True, stop=True)
            nc.tensor.matmul(scr[:1, :], iden_t[:, 0:1], iden_t[:, 0:2],
                             start=True, stop=True)
            nc.tensor.matmul(scr[:, :], onesd_t[:], onesd_t[:, 0:2],
                             start=True, stop=True)
            nc.tensor.matmul(scr[:1, :], bneg_t[:, 0:1], bneg_t[:, 0:2],
                             start=True, stop=True)
            nc.tensor.matmul(scr[:1, :], rmask_t[:, 0:1], rmask_t[:, 0:2],
                             start=True, stop=True)

        q_ta = main_pool.tile([P, NT * NV], f16, tag="qa")
        nc.vector.memset(q_ta[:], 0.0)
        q_tb = main_pool.tile([P, NT * NV], f16, tag="qb")
        nc.vector.memset(q_tb[:], 0.0)
        q3a = q_ta[:].rearrange("p (t v) -> p t v", v=NV)
        q3b = q_tb[:].rearrange("p (t v) -> p t v", v=NV)

        w_all = main_pool.tile([P, 12 * WT], f16, tag="wall")
        e_pool = ctx.enter_context(tc.tile_pool(name="E", bufs=2))
        ln_pool = ctx.enter_context(tc.tile_pool(name="ln", bufs=2))

        # ---------------- w-map precompute (column-sharded) ----------------
        # d-bank is persistent (shared by init + iteration tails).
        d_pool = ctx.enter_context(tc.tile_pool(name="dps", bufs=1,
                                                space="PSUM"))
        DMA_TAPS = range(6, 12)    # replicated via DRAM-bounce DMA reads
        COMP_TAPS = range(0, 6)    # replicated via PE matmul + ACT/DVE copies
        with tc.tile_pool(name="pre", bufs=1) as prep, \
             tc.tile_pool(name="pre2", bufs=2) as prep2, \
             tc.tile_pool(name="psp", bufs=1, space="PSUM") as psp, \
             tc.tile_pool(name="bcp", bufs=2, space="PSUM") as bcp, \
             tc.tile_pool(name="izp", bufs=2, space="PSUM") as izp:
            img_t = prep.tile([PRE_P, IM_U * IM_V], f16, tag="img")
            nc.sync.dma_start(img_t[:], img_d.ap())
            img3 = img_t[:].rearrange("p (u v) -> p u v", v=IM_V)

            def do_tap(ki):
                dy, dx = TAPS[ki]
                diff_t = prep2.tile([PRE_P, KT], f16, tag="diff")
                diff3 = diff_t[:].rearrange("p (t v) -> p t v", v=SS)
                nc.vector.tensor_sub(
                    diff3[:, :, :],
                    img3[:, 2 + dy:2 + dy + NT, 2 + dx:2 + dx + SS],
                    img3[:, 2:2 + NT, 2:2 + SS],
                )
                sq_t = prep2.tile([PRE_P, KT], f16, tag="sq")
                nc.vector.tensor_mul(sq_t[:], diff_t[:], diff_t[:])
                ctap_t = prep2.tile([PRE_P // 3, KT], f16, tag="ctap")
                for cc in range(3):
                    sl = slice(cc * CPQ, (cc + 1) * CPQ)
                    d2_ps = psp.tile([G * NQ, CPQ], f32, tag="d2")
                    nc.tensor.matmul(d2_ps[:], rmask_t[:], sq_t[:, sl],
                                     start=True, stop=True)
                    nc.scalar.activation(ctap_t[:, sl], d2_ps[:], AF.Exp,
                                         scale=-50.0,
                                         bias=lns_t[0:24, ki:ki + 1])
                if ki in DMA_TAPS:
                    nc.sync.dma_start(
                        wbounce_d.ap()[:, ki * KT:(ki + 1) * KT], ctap_t[:])
                else:
                    # PE replication [(qt,g) -> (g,c)] with K=24 masked
                    # stationaries (col-block qt active for rows (qt,*)).
                    for qt in range(NQ):
                        st = st4_t[:, qt * P:(qt + 1) * P]
                        for cc, (o0, o1) in enumerate(
                                ((0, 512), (512, 1024), (1024, KT))):
                            b_ps = bcp.tile([P, 512], f32, tag="bc",
                                            name="b_ps")
                            nc.tensor.matmul(b_ps[:, 0:o1 - o0], st,
                                             ctap_t[:, o0:o1],
                                             start=True, stop=True)
                            wdst = w_all[:, (qt * 12 + ki) * KT + o0:
                                         (qt * 12 + ki) * KT + o1]
                            if (qt + cc) % 2 == 0:
                                nc.scalar.copy(wdst, b_ps[:, 0:o1 - o0])
                            else:
                                nc.vector.tensor_copy(wdst,
                                                      b_ps[:, 0:o1 - o0])

            for ki in list(DMA_TAPS) + list(COMP_TAPS):
                do_tap(ki)
            # DMA-half broadcast via DRAM bounce, grouped (ch, tap-triple)
            # so each start is [6 parts, 4 qt-blocks of 7776 B].
            for ch in range(C):
                for k0 in (6, 9):
                    src_ap = wbounce_d.ap().rearrange(
                        "(q g) (k f) -> g q k f", g=G, f=KT)[
                        :, :, k0:k0 + 3, :]
                    wall_g = w_all[:].rearrange(
                        "(g c) (q k f) -> g c q k f", c=C, k=12, f=KT)
                    nc.sync.dma_start(
                        wall_g[:, ch, :, k0:k0 + 3, :], src_ap)

            # init: q0 = softmax(logits), rotating PSUM banks (z pool is
            # not allocated yet -- front pools own the banks).
            lg3i = lg_t[:].rearrange("p (r x) -> p r x", x=W)
            for c in range(NCH):
                iz = izp.tile([P, CH], f32, tag="iz", name="iz")
                nc.tensor.matmul(iz[:], iden_t[:],
                                 lg2_t[:, c * CH:(c + 1) * CH],
                                 start=True, stop=False,
                                 skip_group_check=True)
                e_t = e_pool.tile([P, CH], f16, tag="E")
                nc.scalar.activation(e_t[:], iz[:], AF.Exp)
                d_ps = d_pool.tile([G, CH], f32, tag="D")
                nc.tensor.matmul(d_ps[:], onesd_t[:], e_t[:],
                                 start=True, stop=True)
                ln_t = ln_pool.tile([G, CH], f16, tag="ln")
                nc.scalar.activation(ln_t[:], d_ps[:], AF.Ln)
                nc.tensor.matmul(iz[:], bneg_t[:], ln_t[:],
                                 start=False, stop=True,
                                 skip_group_check=True)
                nc.scalar.activation(q3a[:, 2 + 2 * c:4 + 2 * c, 2:2 + W],
                                     iz[:], AF.Exp)
                if c == 0:
                    nc.sync.dma_start(q3a[0:105, 16:18, 0:NV],
                                      q3a[21:126, 2:4, 0:NV])
                elif c == 6:
                    nc.sync.dma_start(q3a[21:126, 0:2, 0:NV],
                                      q3a[0:105, 14:16, 0:NV])

        # ---------------- iteration machinery ----------------
        z_pool = ctx.enter_context(tc.tile_pool(name="zps", bufs=1,
                                                space="PSUM"))
        t_pool = ctx.enter_context(tc.tile_pool(name="tprod", bufs=3))

        def z_banks():
            return [z_pool.tile([P, CH], f32, tag=f"z{c}", name=f"z{c}")
                    for c in range(NCH)]

        lg3 = lg_t[:].rearrange("p (r x) -> p r x", x=W)

        def tail(zs, chunks, q3n, last):
            """exp -> D-reduce -> ln -> -lnD broadcast -> exp(q).
            Halo-refresh DMAs for the just-written q fire right after the
            producing chunk (0 -> up-halo, 6 -> down-halo)."""
            for c in chunks:
                e_t = e_pool.tile([P, CH], f16, tag="E")
                nc.scalar.activation(e_t[:], zs[c][:], AF.Exp)
                d_ps = d_pool.tile([G, CH], f32, tag="D")
                nc.tensor.matmul(d_ps[:], onesd_t[:], e_t[:],
                                 start=True, stop=True)
                ln_t = ln_pool.tile([G, CH], f16, tag="ln")
                nc.scalar.activation(ln_t[:], d_ps[:], AF.Ln)
                nc.tensor.matmul(zs[c][:], bneg_t[:], ln_t[:],
                                 start=False, stop=True,
                                 skip_group_check=True)
                if last:
                    nc.scalar.activation(lg3[:, 2 * c:2 * c + 2, 0:W],
                                         zs[c][:], AF.Exp)
                else:
                    nc.scalar.activation(
                        q3n[:, 2 + 2 * c:4 + 2 * c, 2:2 + W], zs[c][:],
                        AF.Exp)
                    if c == 0:
                        nc.sync.dma_start(q3n[0:105, 16:18, 0:NV],
                                          q3n[21:126, 2:4, 0:NV])
                    elif c == 6:
                        nc.sync.dma_start(q3n[21:126, 0:2, 0:NV],
                                          q3n[0:105, 14:16, 0:NV])

        # two row-halves per iteration: tails of half A hide under half B's
        # DVE/PE work; ping-pong q (read old, write new) makes that legal.
        HALVES = [(0, 8, (0, 1, 2, 3)), (8, RG, (6, 4, 5))]

        for it in range(NUM_ITERS):
            last = it == NUM_ITERS - 1
            q3o, q3n = (q3a, q3b) if it % 2 == 0 else (q3b, q3a)
            zs = z_banks()
            for (r0, r1, chunks) in HALVES:
                nr = r1 - r0
                for c in chunks:
                    sl = slice(c * CH, (c + 1) * CH)
                    nc.tensor.matmul(zs[c][:], iden_t[:], lg2_t[:, sl],
                                     start=True, stop=False,
                                     skip_group_check=True)
                    nc.tensor.matmul(zs[c][:], mneg2_t[:],
                                     q3o[:, 2 + 2 * c:4 + 2 * c, 2:2 + W],
                                     start=False, stop=False,
                                     skip_group_check=True)
                for ki, (dy, dx) in enumerate(TAPS):
                    w4 = w_all[:].rearrange(
                        "p (q k t v) -> p q k t v", q=NQ, k=12,
                        v=SS)[:, :, ki]
                    for (qdy, qdx, wdy, wdx) in ((dy, dx, 0, 0),
                                                 (-dy, -dx, -dy, -dx)):
                        q_ap = q3o[:, 2 + qdy + r0:2 + qdy + r1,
                                   2 + qdx:2 + qdx + W].rearrange(
                            "p r (q x) -> p r q x", x=XW)
                        w_ap = w4[:, :, 2 + wdy + r0:2 + wdy + r1,
                                  2 + wdx:2 + wdx + XW].rearrange(
                            "p q r x -> p r q x")
                        t_t = t_pool.tile([P, nr * W], f16, tag=f"t{r0}",
                                          name="t_t")
                        t4 = t_t[:].rearrange("p (r q x) -> p r q x",
                                              q=NQ, x=XW)
                        nc.vector.tensor_mul(t4[:, :, :, :], q_ap, w_ap)
                        for c in chunks:
                            sl = slice((2 * c - r0) * W, (2 * c + 2 - r0) * W)
                            nc.tensor.matmul(
                                zs[c][:], mneg_t[:], t_t[:, sl],
                                start=False, stop=False,
                                skip_group_check=True)
                tail(zs, chunks, q3n, last=last)
                if last:
                    r2 = slice(r0 * W, r1 * W)
                    nc.sync.dma_start(qout_d.ap()[:, r2], lg_t[:, r2])

    _legalize_matmul_waits(nc, mybir)
    return nc


def _legalize_matmul_waits(nc, mybir, max_waits=2):
    """TRN2 ISA sync-wait structs hold few waits per instruction; codegen
    aborts on more. Move excess waits onto InstNoOps (1 wait each) inserted
    right before on the same engine."""
    cap = {}
    for f in nc.m.functions:
        for blk in f.blocks:
            insts = blk.instructions
            out = []
            changed = False
            for i in insts:
                si = getattr(i, "sync_info", None)
                eng = getattr(i, "engine", None)
                max_waits = cap.get(type(i).__name__, 1)
                if (si is not None and eng is not None
                        and len(si.on_wait) > max_waits):
                    waits = list(si.on_wait)
                    keep, move = [], []
                    for w in waits:
                        if "PE" in w.ant_name and len(keep) < max_waits:
                            keep.append(w)
                        else:
                            move.append(w)
                    while len(keep) < max_waits and move:
                        keep.append(move.pop())
                    nop_cap = cap.get("InstNoOp", 1)
                    while move:
                        grp, move = move[:nop_cap], move[nop_cap:]
                        nop = mybir.InstNoOp(
                            name=nc.get_next_instruction_name(),
                            engine=eng, ins=[], outs=[])
                        nop.sync_info = mybir.SyncInfo(on_wait=grp,
                                                       on_update=[])
                        out.append(nop)
                    i.sync_info = mybir.SyncInfo(
                        on_wait=keep, on_update=list(si.on_update))
                    changed = True
                out.append(i)
            if changed:
                blk.instructions = out


def _prep_shards(logits, img, compat):
    """Host-side shard prep -> list of 8 in_maps."""
    mneg = np.kron(np.eye(G), -compat.T.astype(np.float64)).astype(np.float16)
    mneg2 = ((1.0 + WC) * np.kron(np.eye(G), -compat.T.astype(np.float64))
             ).astype(np.float16)
    iden = np.eye(P, dtype=np.float16)
    st4 = np.zeros((G * NQ, NQ * P), np.float16)
    for qt in range(NQ):
        for g in range(G):
            st4[qt * G + g, qt * P + g * C:qt * P + (g + 1) * C] = 1.0
    onesd = np.kron(np.eye(G), np.ones((C, 1))).astype(np.float16)
    bneg = np.kron(np.eye(G), -np.ones((1, C))).astype(np.float16)
    # rmask [(g,rgb,q), (g,q)]: sums rgb
    rmask = np.zeros((PRE_P, G * NQ), np.float16)
    for g in range(G):
        for rgb in range(3):
            for qt in range(NQ):
                rmask[(g * 3 + rgb) * NQ + qt, qt * G + g] = 1.0
    lns = np.tile(
        np.array([math.log(SW[2 + dy, 2 + dx]) for (dy, dx) in TAPS],
                 np.float32)[None, :], (G * NQ, 1))

    in_maps = []
    for core in range(8):
        b, j = divmod(core, 4)
        s = STARTS[j]
        lg = logits[b, :, s:s + 84, :].reshape(C, G, RG, W)
        lg = np.ascontiguousarray(
            lg.transpose(1, 0, 2, 3).reshape(P, F)).astype(np.float32)
        im = np.zeros((G, 3, NQ, IM_U, IM_V), np.float32)
        for g in range(G):
            rbase = s + g * RG - 4
            u0, u1 = max(0, -rbase), min(IM_U, H - rbase)
            for qt in range(NQ):
                cbase = qt * XW - 4
                v0, v1 = max(0, -cbase), min(IM_V, W - cbase)
                im[g, :, qt, u0:u1, v0:v1] = img[
                    b, :, rbase + u0:rbase + u1, cbase + v0:cbase + v1]
        im = im.reshape(PRE_P, IM_U * IM_V).astype(np.float16)
        in_maps.append({
            "lg": lg, "img": np.ascontiguousarray(im),
            "mneg": mneg, "mneg2": mneg2, "st4": st4, "iden": iden, "onesd": onesd,
            "bneg": bneg, "rmask": rmask, "lns": lns,
        })
    return in_maps


def kernel(**inputs):
    logits = np.asarray(inputs["logits"], dtype=np.float32)
    img = np.asarray(inputs["img"], dtype=np.float32)
    compat = np.asarray(inputs["compat_mat"], dtype=np.float32)

    from concourse.bass_utils import run_bass_kernel_spmd
    if "nc" not in _BASS_CACHE:
        _BASS_CACHE["nc"] = _build_bass()
    nc = _BASS_CACHE["nc"]

    in_maps = _prep_shards(logits, img, compat)
    res = run_bass_kernel_spmd(nc, in_maps, core_ids=list(range(8)))
    _BASS_CACHE["last_result"] = res

    out = np.zeros((B, C, H, W), np.float32)
    for core in range(8):
        b, j = divmod(core, 4)
        s = STARTS[j]
        lo, hi = OWN[j]
        qc = res.results[core]["qout"].reshape(G, C, RG, W)
        qc = qc.transpose(1, 0, 2, 3).reshape(C, 84, W)
        out[b, :, s + lo:s + hi, :] = qc[:, lo:hi, :]
    return out


# revision 29
# speedup vs baseline: 1.0385x; 1.0037x over previous
"""CRF-as-RNN mean-field kernel for Trainium2 (Bass/Tile), 8-core SPMD.

Strategy (v2):
- Shard 2 images x 4 row-strips across 8 cores; 84 rows/core (64 owned +
  halo), 5 mean-field iterations shrink validity, no inter-core comms.
- Partitions = 6 row-groups x 21 channels = 126; free = 14 rows x 256 cols
  (+2 halos: 18 x 260 slots).
- The 5x5 spatial gaussian (sigma=0.1) is a delta => sp == q, folded into
  a second stationary mneg2 = (1+wc) * mneg applied to q via PE.
- Bilateral: 24 shifted products on DVE (fp16 2x), accumulated ON THE PE
  via mneg x t_k matmuls into 7 persistent PSUM z-banks (compat transform
  is linear). No DVE adds, no SBUF acc tile, f32 accumulation.
- Softmax: z-banks already hold logits + pairwise (logits fed as f32
  bitcast to f32r: full PE rate at 512 cols). exp/ln on ACT; lnD
  broadcast back via bneg (f32r) matmul; final exp writes q (fp16) or the
  f32 output tile.
- w-precompute, column-sharded: partitions (group, rgb, quarter) = 72;
  per tap: DVE diff, DVE square, PE rgb-reduce -> [24, 390] PSUM x3,
  ACT exp(-50*d2 + ln(spatial)) -> compact [24, 12*1170] fp16; then 84
  strided-partition DMAs replicate rows to the 21 channel partitions
  (w_all [126, 12*4680] fp16).
"""

import math
import sys
from contextlib import ExitStack

import numpy as np

sys.path.insert(0, "/opt/trn_rl_repo")

# ---------------- problem constants (hardcoded per contract) ----------------
B, C, H, W = 2, 21, 256, 256
G, RG = 6, 14                  # row groups per strip, rows per group
P = G * C                      # 126 partitions
F = RG * W                     # 3584 free elems per partition
NT, NV = 18, 260               # q/w map slots: rows -2..15, cols -2..257
STARTS = [0, 54, 118, 172]     # strip start rows
OWN = [(0, 64), (10, 74), (10, 74), (20, 84)]  # owned local-row range
NUM_ITERS = 5
NCH, CH = 7, 512               # softmax chunks (512 px = 2 rows)
NQ, XW, SS = 4, 64, 72         # col chunks: 4 x 64 owned px, 72 stored slots
IM_U, IM_V = 22, 77            # img chunk slots: rows -4..17, 77 cols
KT = NT * SS                   # 1296 map elems per (tap, chunk)
WT = NQ * KT                   # 5184 w elems per tap
PRE_P = G * 3 * NQ             # 72 precompute partitions (g, rgb, chunk)
CPQ = 432                      # precompute PSUM chunk (3 x 432 = 1296)

# spatial gaussian (5x5, sigma=5), normalized
_ax = np.arange(5, dtype=np.float64) - 2
_xx, _yy = np.meshgrid(_ax, _ax, indexing="ij")
_g = np.exp(-(_xx**2 + _yy**2) / (2 * 5.0**2))
SW = (_g / _g.sum()).astype(np.float64)
WC = float(SW[2, 2])           # center weight (spatial only; color=1)
# 12 unique taps (positive half-window); opposite taps share weight maps.
# dy=0 taps first: their muls (and mirrors) don't read halo rows, covering
# the intra-iteration halo-refresh DMA latency.
TAPS = [(0, 1), (0, 2), (1, -2), (1, -1), (1, 0), (1, 1), (1, 2),
        (2, -2), (2, -1), (2, 0), (2, 1), (2, 2)]

_BASS_CACHE = {}


def _build_bass():
    import concourse.bass as bass
    import concourse.mybir as mybir
    from concourse import tile

    f32 = mybir.dt.float32
    f32r = mybir.dt.float32r
    f16 = mybir.dt.float16
    AF = mybir.ActivationFunctionType
    OP = mybir.AluOpType

    nc = bass.Bass("TRN2", target_bir_lowering=False, debug=False,
                   enable_asserts=False)

    lg_d = nc.dram_tensor("lg", [P, F], f32, kind="ExternalInput")
    img_d = nc.dram_tensor("img", [PRE_P, IM_U * IM_V], f16,
                           kind="ExternalInput")
    mneg_d = nc.dram_tensor("mneg", [P, P], f16, kind="ExternalInput")
    mneg2_d = nc.dram_tensor("mneg2", [P, P], f16, kind="ExternalInput")
    st4_d = nc.dram_tensor("st4", [G * NQ, NQ * P], f16,
                            kind="ExternalInput")
    iden_d = nc.dram_tensor("iden", [P, P], f16, kind="ExternalInput")
    onesd_d = nc.dram_tensor("onesd", [P, G], f16, kind="ExternalInput")
    bneg_d = nc.dram_tensor("bneg", [G, P], f16, kind="ExternalInput")
    rmask_d = nc.dram_tensor("rmask", [PRE_P, G * NQ], f16,
                             kind="ExternalInput")
    lns_d = nc.dram_tensor("lns", [G * NQ, 12], f32, kind="ExternalInput")
    wbounce_d = nc.dram_tensor("wbounce", [PRE_P // 3, 12 * KT], f16,
                               kind="Internal")
    qout_d = nc.dram_tensor("qout", [P, F], f32, kind="ExternalOutput")

    with tile.TileContext(nc) as tc, ExitStack() as ctx:
        const_pool = ctx.enter_context(tc.tile_pool(name="const", bufs=1))
        main_pool = ctx.enter_context(tc.tile_pool(name="main", bufs=1))

        mneg_t = const_pool.tile([P, P], f16, tag="mneg")
        nc.sync.dma_start(mneg_t[:], mneg_d.ap())
        mneg2_t = const_pool.tile([P, P], f16, tag="mneg2")
        nc.sync.dma_start(mneg2_t[:], mneg2_d.ap())
        st4_t = const_pool.tile([G * NQ, NQ * P], f16, tag="st4")
        nc.sync.dma_start(st4_t[:], st4_d.ap())
        iden_t = const_pool.tile([P, P], f16, tag="iden")
        nc.sync.dma_start(iden_t[:], iden_d.ap())
        onesd_t = const_pool.tile([P, G], f16, tag="onesd")
        nc.sync.dma_start(onesd_t[:], onesd_d.ap())
        bneg_t = const_pool.tile([G, P], f16, tag="bneg")
        nc.sync.dma_start(bneg_t[:], bneg_d.ap())
        rmask_t = const_pool.tile([PRE_P, G * NQ], f16, tag="rmask")
        nc.sync.dma_start(rmask_t[:], rmask_d.ap())
        lns_t = const_pool.tile([G * NQ, 12], f32, tag="lns")
        nc.sync.dma_start(lns_t[:], lns_d.ap())

        lg_t = main_pool.tile([P, F], f32, tag="lg")
        nc.sync.dma_start(lg_t[:], lg_d.ap())
        lg2_t = main_pool.tile([P, F], f16, tag="lg2")
        nc.scalar.copy(lg2_t[:], lg_t[:])

        # Absorber matmuls: pre-observe every stationary's DMA queue with a
        # tiny dummy matmul so real matmuls carry at most 1 extra wait.
        with tc.tile_pool(name="scrp", bufs=1, space="PSUM") as scrp:
            scr = scrp.tile([G, 2], f32, tag="scr")
            nc.tensor.matmul(scr[:1, :], mneg_t[:, 0:1], mneg_t[:, 0:2],
                             start=True, stop=True)
            nc.tensor.matmul(scr[:1, :], mneg2_t[:, 0:1], mneg2_t[:, 0:2],
                             start=True, stop=True)
            nc.tensor.matmul(scr[:1, :], st4_t[:, 0:1], st4_t[:, 0:2],
                             start=True, stop=True)
            nc.tensor.matmul(scr[:1, :], iden_t[:, 0:1], iden_t[:, 0:2],
                             start=True, stop=True)
            nc.tensor.matmul(scr[:, :], onesd_t[:], onesd_t[:, 0:2],
                             start=True, stop=True)
            nc.tensor.matmul(scr[:1, :], bneg_t[:, 0:1], bneg_t[:, 0:2],
                             start=True, stop=True)
            nc.tensor.matmul(scr[:1, :], rmask_t[:, 0:1], rmask_t[:, 0:2],
                             start=True, stop=True)

        q_ta = main_pool.tile([P, NT * NV], f16, tag="qa")
        nc.vector.memset(q_ta[:], 0.0)
        q_tb = main_pool.tile([P, NT * NV], f16, tag="qb")
        nc.vector.memset(q_tb[:], 0.0)
        q3a = q_ta[:].rearrange("p (t v) -> p t v", v=NV)
        q3b = q_tb[:].rearrange("p (t v) -> p t v", v=NV)

        w_all = main_pool.tile([P, 12 * WT], f16, tag="wall")
        e_pool = ctx.enter_context(tc.tile_pool(name="E", bufs=2))
        ln_pool = ctx.enter_context(tc.tile_pool(name="ln", bufs=2))

        # ---------------- w-map precompute (column-sharded) ----------------
        # d-bank is persistent (shared by init + iteration tails).
        d_pool = ctx.enter_context(tc.tile_pool(name="dps", bufs=1,
                                                space="PSUM"))
        DMA_TAPS = range(6, 12)    # replicated via DRAM-bounce DMA reads
        COMP_TAPS = range(0, 6)    # replicated via PE matmul + ACT/DVE copies
        with tc.tile_pool(name="pre", bufs=1) as prep, \
             tc.tile_pool(name="pre2", bufs=2) as prep2, \
             tc.tile_pool(name="psp", bufs=1, space="PSUM") as psp, \
             tc.tile_pool(name="bcp", bufs=2, space="PSUM") as bcp, \
             tc.tile_pool(name="izp", bufs=2, space="PSUM") as izp:
            img_t = prep.tile([PRE_P, IM_U * IM_V], f16, tag="img")
            nc.sync.dma_start(img_t[:], img_d.ap())
            img3 = img_t[:].rearrange("p (u v) -> p u v", v=IM_V)

            def do_tap(ki):
                dy, dx = TAPS[ki]
                diff_t = prep2.tile([PRE_P, KT], f16, tag="diff")
                diff3 = diff_t[:].rearrange("p (t v) -> p t v", v=SS)
                nc.vector.tensor_sub(
                    diff3[:, :, :],
                    img3[:, 2 + dy:2 + dy + NT, 2 + dx:2 + dx + SS],
                    img3[:, 2:2 + NT, 2:2 + SS],
                )
                sq_t = prep2.tile([PRE_P, KT], f16, tag="sq")
                nc.vector.tensor_mul(sq_t[:], diff_t[:], diff_t[:])
                ctap_t = prep2.tile([PRE_P // 3, KT], f16, tag="ctap")
                for cc in range(3):
                    sl = slice(cc * CPQ, (cc + 1) * CPQ)
                    d2_ps = psp.tile([G * NQ, CPQ], f32, tag="d2")
                    nc.tensor.matmul(d2_ps[:], rmask_t[:], sq_t[:, sl],
                                     start=True, stop=True)
                    nc.scalar.activation(ctap_t[:, sl], d2_ps[:], AF.Exp,
                                         scale=-50.0,
                                         bias=lns_t[0:24, ki:ki + 1])
                if ki in DMA_TAPS:
                    nc.sync.dma_start(
                        wbounce_d.ap()[:, ki * KT:(ki + 1) * KT], ctap_t[:])
                else:
                    # PE replication [(qt,g) -> (g,c)] with K=24 masked
                    # stationaries (col-block qt active for rows (qt,*)).
                    for qt in range(NQ):
                        st = st4_t[:, qt * P:(qt + 1) * P]
                        for cc, (o0, o1) in enumerate(
                                ((0, 512), (512, 1024), (1024, KT))):
                            b_ps = bcp.tile([P, 512], f32, tag="bc",
                                            name="b_ps")
                            nc.tensor.matmul(b_ps[:, 0:o1 - o0], st,
                                             ctap_t[:, o0:o1],
                                             start=True, stop=True)
                            wdst = w_all[:, (qt * 12 + ki) * KT + o0:
                                         (qt * 12 + ki) * KT + o1]
                            if (qt + cc) % 2 == 0:
                                nc.scalar.copy(wdst, b_ps[:, 0:o1 - o0])
                            else:
                                nc.vector.tensor_copy(wdst,
                                                      b_ps[:, 0:o1 - o0])

            for ki in list(DMA_TAPS) + list(COMP_TAPS):
                do_tap(ki)
            # DMA-half broadcast via DRAM bounce, grouped (ch, tap-triple)
            # so each start is [6 parts, 4 qt-blocks of 7776 B].
            for ch in range(C):
                for k0 in (6, 9):
                    src_ap = wbounce_d.ap().rearrange(
                        "(q g) (k f) -> g q k f", g=G, f=KT)[
                        :, :, k0:k0 + 3, :]
                    wall_g = w_all[:].rearrange(
                        "(g c) (q k f) -> g c q k f", c=C, k=12, f=KT)
                    nc.sync.dma_start(
                        wall_g[:, ch, :, k0:k0 + 3, :], src_ap)

            # init: q0 = softmax(logits), rotating PSUM banks (z pool is
            # not allocated yet -- front pools own the banks).
            lg3i = lg_t[:].rearrange("p (r x) -> p r x", x=W)
            for c in range(NCH):
                iz = izp.tile([P, CH], f32, tag="iz", name="iz")
                nc.tensor.matmul(iz[:], iden_t[:],
                                 lg2_t[:, c * CH:(c + 1) * CH],
                                 start=True, stop=False,
                                 skip_group_check=True)
                e_t = e_pool.tile([P, CH], f16, tag="E")
                nc.scalar.activation(e_t[:], iz[:], AF.Exp)
                d_ps = d_pool.tile([G, CH], f32, tag="D")
                nc.tensor.matmul(d_ps[:], onesd_t[:], e_t[:],
                                 start=True, stop=True)
                ln_t = ln_pool.tile([G, CH], f16, tag="ln")
                nc.scalar.activation(ln_t[:], d_ps[:], AF.Ln)
                nc.tensor.matmul(iz[:], bneg_t[:], ln_t[:],
                                 start=False, stop=True,
                                 skip_group_check=True)
                nc.scalar.activation(q3a[:, 2 + 2 * c:4 + 2 * c, 2:2 + W],
                                     iz[:], AF.Exp)
                if c == 0:
                    nc.sync.dma_start(q3a[0:105, 16:18, 0:NV],
                                      q3a[21:126, 2:4, 0:NV])
                elif c == 6:
                    nc.sync.dma_start(q3a[21:126, 0:2, 0:NV],
                                      q3a[0:105, 14:16, 0:NV])

        # ---------------- iteration machinery ----------------
        z_pool = ctx.enter_context(tc.tile_pool(name="zps", bufs=1,
                                                space="PSUM"))
        t_pool = ctx.enter_context(tc.tile_pool(name="tprod", bufs=3))

        def z_banks():
            return [z_pool.tile([P, CH], f32, tag=f"z{c}", name=f"z{c}")
                    for c in range(NCH)]

        lg3 = lg_t[:].rearrange("p (r x) -> p r x", x=W)

        def tail(zs, chunks, q3n, last):
            """exp -> D-reduce -> ln -> -lnD broadcast -> exp(q).
            Halo-refresh DMAs for the just-written q fire right after the
            producing chunk (0 -> up-halo, 6 -> down-halo)."""
            for c in chunks:
                e_t = e_pool.tile([P, CH], f16, tag="E")
                nc.scalar.activation(e_t[:], zs[c][:], AF.Exp)
                d_ps = d_pool.tile([G, CH], f32, tag="D")
                nc.tensor.matmul(d_ps[:], onesd_t[:], e_t[:],
                                 start=True, stop=True)
                ln_t = ln_pool.tile([G, CH], f16, tag="ln")
                nc.scalar.activation(ln_t[:], d_ps[:], AF.Ln)
                nc.tensor.matmul(zs[c][:], bneg_t[:], ln_t[:],
                                 start=False, stop=True,
                                 skip_group_check=True)
                if last:
                    nc.scalar.activation(lg3[:, 2 * c:2 * c + 2, 0:W],
                                         zs[c][:], AF.Exp)
                    r2 = slice(2 * c * W, (2 * c + 2) * W)
                    nc.sync.dma_start(qout_d.ap()[:, r2], lg_t[:, r2])
                else:
                    nc.scalar.activation(
                        q3n[:, 2 + 2 * c:4 + 2 * c, 2:2 + W], zs[c][:],
                        AF.Exp)
                    if c == 0:
                        nc.sync.dma_start(q3n[0:105, 16:18, 0:NV],
                                          q3n[21:126, 2:4, 0:NV])
                    elif c == 6:
                        nc.sync.dma_start(q3n[21:126, 0:2, 0:NV],
                                          q3n[0:105, 14:16, 0:NV])

        # two row-halves per iteration: tails of half A hide under half B's
        # DVE/PE work; ping-pong q (read old, write new) makes that legal.
        HALVES = [(0, 8, (0, 1, 2, 3)), (8, RG, (6, 4, 5))]

        for it in range(NUM_ITERS):
            last = it == NUM_ITERS - 1
            q3o, q3n = (q3a, q3b) if it % 2 == 0 else (q3b, q3a)
            zs = z_banks()
            for (r0, r1, chunks) in HALVES:
                nr = r1 - r0
                for c in chunks:
                    sl = slice(c * CH, (c + 1) * CH)
                    nc.tensor.matmul(zs[c][:], iden_t[:], lg2_t[:, sl],
                                     start=True, stop=False,
                                     skip_group_check=True)
                    nc.tensor.matmul(zs[c][:], mneg2_t[:],
                                     q3o[:, 2 + 2 * c:4 + 2 * c, 2:2 + W],
                                     start=False, stop=False,
                                     skip_group_check=True)
                for ki, (dy, dx) in enumerate(TAPS):
                    w4 = w_all[:].rearrange(
                        "p (q k t v) -> p q k t v", q=NQ, k=12,
                        v=SS)[:, :, ki]
                    for (qdy, qdx, wdy, wdx) in ((dy, dx, 0, 0),
                                                 (-dy, -dx, -dy, -dx)):
                        q_ap = q3o[:, 2 + qdy + r0:2 + qdy + r1,
                                   2 + qdx:2 + qdx + W].rearrange(
                            "p r (q x) -> p r q x", x=XW)
                        w_ap = w4[:, :, 2 + wdy + r0:2 + wdy + r1,
                                  2 + wdx:2 + wdx + XW].rearrange(
                            "p q r x -> p r q x")
                        t_t = t_pool.tile([P, nr * W], f16, tag=f"t{r0}",
                                          name="t_t")
                        t4 = t_t[:].rearrange("p (r q x) -> p r q x",
                                              q=NQ, x=XW)
                        nc.vector.tensor_mul(t4[:, :, :, :], q_ap, w_ap)
                        for c in chunks:
                            sl = slice((2 * c - r0) * W, (2 * c + 2 - r0) * W)
                            nc.tensor.matmul(
                                zs[c][:], mneg_t[:], t_t[:, sl],
                                start=False, stop=False,
                                skip_group_check=True)
                tail(zs, chunks, q3n, last=last)

    _legalize_matmul_waits(nc, mybir)
    return nc


def _legalize_matmul_waits(nc, mybir, max_waits=2):
    """TRN2 ISA sync-wait structs hold few waits per instruction; codegen
    aborts on more. Move excess waits onto InstNoOps (1 wait each) inserted
    right before on the same engine."""
    cap = {}
    for f in nc.m.functions:
        for blk in f.blocks:
            insts = blk.instructions
            out = []
            changed = False
            for i in insts:
                si = getattr(i, "sync_info", None)
                eng = getattr(i, "engine", None)
                max_waits = cap.get(type(i).__name__, 1)
                if (si is not None and eng is not None
                        and len(si.on_wait) > max_waits):
                    waits = list(si.on_wait)
                    keep, move = [], []
                    for w in waits:
                        if "PE" in w.ant_name and len(keep) < max_waits:
                            keep.append(w)
                        else:
                            move.append(w)
                    while len(keep) < max_waits and move:
                        keep.append(move.pop())
                    nop_cap = cap.get("InstNoOp", 1)
                    while move:
                        grp, move = move[:nop_cap], move[nop_cap:]
                        nop = mybir.InstNoOp(
                            name=nc.get_next_instruction_name(),
                            engine=eng, ins=[], outs=[])
                        nop.sync_info = mybir.SyncInfo(on_wait=grp,
                                                       on_update=[])
                        out.append(nop)
                    i.sync_info = mybir.SyncInfo(
                        on_wait=keep, on_update=list(si.on_update))
                    changed = True
                out.append(i)
            if changed:
                blk.instructions = out


def _prep_shards(logits, img, compat):
    """Host-side shard prep -> list of 8 in_maps."""
    mneg = np.kron(np.eye(G), -compat.T.astype(np.float64)).astype(np.float16)
    mneg2 = ((1.0 + WC) * np.kron(np.eye(G), -compat.T.astype(np.float64))
             ).astype(np.float16)
    iden = np.eye(P, dtype=np.float16)
    st4 = np.zeros((G * NQ, NQ * P), np.float16)
    for qt in range(NQ):
        for g in range(G):
            st4[qt * G + g, qt * P + g * C:qt * P + (g + 1) * C] = 1.0
    onesd = np.kron(np.eye(G), np.ones((C, 1))).astype(np.float16)
    bneg = np.kron(np.eye(G), -np.ones((1, C))).astype(np.float16)
    # rmask [(g,rgb,q), (g,q)]: sums rgb
    rmask = np.zeros((PRE_P, G * NQ), np.float16)
    for g in range(G):
        for rgb in range(3):
            for qt in range(NQ):
                rmask[(g * 3 + rgb) * NQ + qt, qt * G + g] = 1.0
    lns = np.tile(
        np.array([math.log(SW[2 + dy, 2 + dx]) for (dy, dx) in TAPS],
                 np.float32)[None, :], (G * NQ, 1))

    in_maps = []
    for core in range(8):
        b, j = divmod(core, 4)
        s = STARTS[j]
        lg = logits[b, :, s:s + 84, :].reshape(C, G, RG, W)
        lg = np.ascontiguousarray(
            lg.transpose(1, 0, 2, 3).reshape(P, F)).astype(np.float32)
        im = np.zeros((G, 3, NQ, IM_U, IM_V), np.float32)
        for g in range(G):
            rbase = s + g * RG - 4
            u0, u1 = max(0, -rbase), min(IM_U, H - rbase)
            for qt in range(NQ):
                cbase = qt * XW - 4
                v0, v1 = max(0, -cbase), min(IM_V, W - cbase)
                im[g, :, qt, u0:u1, v0:v1] = img[
                    b, :, rbase + u0:rbase + u1, cbase + v0:cbase + v1]
        im = im.reshape(PRE_P, IM_U * IM_V).astype(np.float16)
        in_maps.append({
            "lg": lg, "img": np.ascontiguousarray(im),
            "mneg": mneg, "mneg2": mneg2, "st4": st4, "iden": iden, "onesd": onesd,
            "bneg": bneg, "rmask": rmask, "lns": lns,
        })
    return in_maps


def kernel(**inputs):
    logits = np.asarray(inputs["logits"], dtype=np.float32)
    img = np.asarray(inputs["img"], dtype=np.float32)
    compat = np.asarray(inputs["compat_mat"], dtype=np.float32)

    from concourse.bass_utils import run_bass_kernel_spmd
    if "nc" not in _BASS_CACHE:
        _BASS_CACHE["nc"] = _build_bass()
    nc = _BASS_CACHE["nc"]

    in_maps = _prep_shards(logits, img, compat)
    res = run_bass_kernel_spmd(nc, in_maps, core_ids=list(range(8)))
    _BASS_CACHE["last_result"] = res

    out = np.zeros((B, C, H, W), np.float32)
    for core in range(8):
        b, j = divmod(core, 4)
        s = STARTS[j]
        lo, hi = OWN[j]
        qc = res.results[core]["qout"].reshape(G, C, RG, W)
        qc = qc.transpose(1, 0, 2, 3).reshape(C, 84, W)
        out[b, :, s + lo:s + hi, :] = qc[:, lo:hi, :]
    return out


# revision 30
# speedup vs baseline: 1.0534x; 1.0143x over previous
"""CRF-as-RNN mean-field kernel for Trainium2 (Bass/Tile), 8-core SPMD.

Strategy (v2):
- Shard 2 images x 4 row-strips across 8 cores; 84 rows/core (64 owned +
  halo), 5 mean-field iterations shrink validity, no inter-core comms.
- Partitions = 6 row-groups x 21 channels = 126; free = 14 rows x 256 cols
  (+2 halos: 18 x 260 slots).
- The 5x5 spatial gaussian (sigma=0.1) is a delta => sp == q, folded into
  a second stationary mneg2 = (1+wc) * mneg applied to q via PE.
- Bilateral: 24 shifted products on DVE (fp16 2x), accumulated ON THE PE
  via mneg x t_k matmuls into 7 persistent PSUM z-banks (compat transform
  is linear). No DVE adds, no SBUF acc tile, f32 accumulation.
- Softmax: z-banks already hold logits + pairwise (logits fed as f32
  bitcast to f32r: full PE rate at 512 cols). exp/ln on ACT; lnD
  broadcast back via bneg (f32r) matmul; final exp writes q (fp16) or the
  f32 output tile.
- w-precompute, column-sharded: partitions (group, rgb, quarter) = 72;
  per tap: DVE diff, DVE square, PE rgb-reduce -> [24, 390] PSUM x3,
  ACT exp(-50*d2 + ln(spatial)) -> compact [24, 12*1170] fp16; then 84
  strided-partition DMAs replicate rows to the 21 channel partitions
  (w_all [126, 12*4680] fp16).
"""

import math
import sys
from contextlib import ExitStack

import numpy as np

sys.path.insert(0, "/opt/trn_rl_repo")

# ---------------- problem constants (hardcoded per contract) ----------------
B, C, H, W = 2, 21, 256, 256
G, RG = 6, 14                  # row groups per strip, rows per group
P = G * C                      # 126 partitions
F = RG * W                     # 3584 free elems per partition
NT, NV = 18, 260               # q/w map slots: rows -2..15, cols -2..257
STARTS = [0, 54, 118, 172]     # strip start rows
OWN = [(0, 64), (10, 74), (10, 74), (20, 84)]  # owned local-row range
NUM_ITERS = 5
NCH, CH = 7, 512               # softmax chunks (512 px = 2 rows)
NQ, XW, SS = 4, 64, 72         # col chunks: 4 x 64 owned px, 72 stored slots
IM_U, IM_V = 22, 77            # img chunk slots: rows -4..17, 77 cols
KT = NT * SS                   # 1296 map elems per (tap, chunk)
WT = NQ * KT                   # 5184 w elems per tap
PRE_P = G * 3 * NQ             # 72 precompute partitions (g, rgb, chunk)
CPQ = 432                      # precompute PSUM chunk (3 x 432 = 1296)

# spatial gaussian (5x5, sigma=5), normalized
_ax = np.arange(5, dtype=np.float64) - 2
_xx, _yy = np.meshgrid(_ax, _ax, indexing="ij")
_g = np.exp(-(_xx**2 + _yy**2) / (2 * 5.0**2))
SW = (_g / _g.sum()).astype(np.float64)
WC = float(SW[2, 2])           # center weight (spatial only; color=1)
# 12 unique taps (positive half-window); opposite taps share weight maps.
# dy=0 taps first: their muls (and mirrors) don't read halo rows, covering
# the intra-iteration halo-refresh DMA latency.
TAPS = [(0, 1), (0, 2), (1, -2), (1, -1), (1, 0), (1, 1), (1, 2),
        (2, -2), (2, -1), (2, 0), (2, 1), (2, 2)]

_BASS_CACHE = {}


def _build_bass():
    import concourse.bass as bass
    import concourse.mybir as mybir
    from concourse import tile

    f32 = mybir.dt.float32
    f32r = mybir.dt.float32r
    f16 = mybir.dt.float16
    AF = mybir.ActivationFunctionType
    OP = mybir.AluOpType

    nc = bass.Bass("TRN2", target_bir_lowering=False, debug=False,
                   enable_asserts=False)

    lg_d = nc.dram_tensor("lg", [P, F], f32, kind="ExternalInput")
    img_d = nc.dram_tensor("img", [PRE_P, IM_U * IM_V], f16,
                           kind="ExternalInput")
    mneg_d = nc.dram_tensor("mneg", [P, P], f16, kind="ExternalInput")
    mneg2_d = nc.dram_tensor("mneg2", [P, P], f16, kind="ExternalInput")
    st4_d = nc.dram_tensor("st4", [G * NQ, NQ * P], f16,
                            kind="ExternalInput")
    iden_d = nc.dram_tensor("iden", [P, P], f16, kind="ExternalInput")
    onesd_d = nc.dram_tensor("onesd", [P, G], f16, kind="ExternalInput")
    bneg_d = nc.dram_tensor("bneg", [G, P], f16, kind="ExternalInput")
    rmask_d = nc.dram_tensor("rmask", [PRE_P, G * NQ], f16,
                             kind="ExternalInput")
    lns_d = nc.dram_tensor("lns", [G * NQ, 12], f32, kind="ExternalInput")
    wbounce_d = nc.dram_tensor("wbounce", [PRE_P // 3, 12 * KT], f16,
                               kind="Internal")
    qout_d = nc.dram_tensor("qout", [P, F], f32, kind="ExternalOutput")

    with tile.TileContext(nc) as tc, ExitStack() as ctx:
        const_pool = ctx.enter_context(tc.tile_pool(name="const", bufs=1))
        main_pool = ctx.enter_context(tc.tile_pool(name="main", bufs=1))

        mneg_t = const_pool.tile([P, P], f16, tag="mneg")
        nc.sync.dma_start(mneg_t[:], mneg_d.ap())
        mneg2_t = const_pool.tile([P, P], f16, tag="mneg2")
        nc.sync.dma_start(mneg2_t[:], mneg2_d.ap())
        st4_t = const_pool.tile([G * NQ, NQ * P], f16, tag="st4")
        nc.sync.dma_start(st4_t[:], st4_d.ap())
        iden_t = const_pool.tile([P, P], f16, tag="iden")
        nc.sync.dma_start(iden_t[:], iden_d.ap())
        onesd_t = const_pool.tile([P, G], f16, tag="onesd")
        nc.sync.dma_start(onesd_t[:], onesd_d.ap())
        bneg_t = const_pool.tile([G, P], f16, tag="bneg")
        nc.sync.dma_start(bneg_t[:], bneg_d.ap())
        rmask_t = const_pool.tile([PRE_P, G * NQ], f16, tag="rmask")
        nc.sync.dma_start(rmask_t[:], rmask_d.ap())
        lns_t = const_pool.tile([G * NQ, 12], f32, tag="lns")
        nc.sync.dma_start(lns_t[:], lns_d.ap())

        lg_t = main_pool.tile([P, F], f32, tag="lg")
        nc.sync.dma_start(lg_t[:], lg_d.ap())
        lg2_t = main_pool.tile([P, F], f16, tag="lg2")
        nc.scalar.copy(lg2_t[:], lg_t[:])

        # Absorber matmuls: pre-observe every stationary's DMA queue with a
        # tiny dummy matmul so real matmuls carry at most 1 extra wait.
        with tc.tile_pool(name="scrp", bufs=1, space="PSUM") as scrp:
            scr = scrp.tile([G, 2], f32, tag="scr")
            nc.tensor.matmul(scr[:1, :], mneg_t[:, 0:1], mneg_t[:, 0:2],
                             start=True, stop=True)
            nc.tensor.matmul(scr[:1, :], mneg2_t[:, 0:1], mneg2_t[:, 0:2],
                             start=True, stop=True)
            nc.tensor.matmul(scr[:1, :], st4_t[:, 0:1], st4_t[:, 0:2],
                             start=True, stop=True)
            nc.tensor.matmul(scr[:1, :], iden_t[:, 0:1], iden_t[:, 0:2],
                             start=True, stop=True)
            nc.tensor.matmul(scr[:, :], onesd_t[:], onesd_t[:, 0:2],
                             start=True, stop=True)
            nc.tensor.matmul(scr[:1, :], bneg_t[:, 0:1], bneg_t[:, 0:2],
                             start=True, stop=True)
            nc.tensor.matmul(scr[:1, :], rmask_t[:, 0:1], rmask_t[:, 0:2],
                             start=True, stop=True)

        q_ta = main_pool.tile([P, NT * NV], f16, tag="qa")
        nc.vector.memset(q_ta[:], 0.0)
        q_tb = main_pool.tile([P, NT * NV], f16, tag="qb")
        nc.vector.memset(q_tb[:], 0.0)
        q3a = q_ta[:].rearrange("p (t v) -> p t v", v=NV)
        q3b = q_tb[:].rearrange("p (t v) -> p t v", v=NV)

        w_all = main_pool.tile([P, 12 * WT], f16, tag="wall")
        e_pool = ctx.enter_context(tc.tile_pool(name="E", bufs=2))
        ln_pool = ctx.enter_context(tc.tile_pool(name="ln", bufs=2))

        # ---------------- w-map precompute (column-sharded) ----------------
        # d-bank is persistent (shared by init + iteration tails).
        d_pool = ctx.enter_context(tc.tile_pool(name="dps", bufs=1,
                                                space="PSUM"))
        DMA_TAPS = range(6, 12)    # replicated via DRAM-bounce DMA reads
        COMP_TAPS = range(0, 6)    # replicated via PE matmul + ACT/DVE copies
        with tc.tile_pool(name="pre", bufs=1) as prep, \
             tc.tile_pool(name="pre2", bufs=3) as prep2, \
             tc.tile_pool(name="psp", bufs=1, space="PSUM") as psp, \
             tc.tile_pool(name="bcp", bufs=2, space="PSUM") as bcp, \
             tc.tile_pool(name="izp", bufs=2, space="PSUM") as izp:
            img_t = prep.tile([PRE_P, IM_U * IM_V], f16, tag="img")
            nc.sync.dma_start(img_t[:], img_d.ap())
            img3 = img_t[:].rearrange("p (u v) -> p u v", v=IM_V)

            def do_tap(ki):
                dy, dx = TAPS[ki]
                diff_t = prep2.tile([PRE_P, KT], f16, tag="diff")
                diff3 = diff_t[:].rearrange("p (t v) -> p t v", v=SS)
                nc.vector.tensor_sub(
                    diff3[:, :, :],
                    img3[:, 2 + dy:2 + dy + NT, 2 + dx:2 + dx + SS],
                    img3[:, 2:2 + NT, 2:2 + SS],
                )
                sq_t = prep2.tile([PRE_P, KT], f16, tag="sq")
                nc.vector.tensor_mul(sq_t[:], diff_t[:], diff_t[:])
                ctap_t = prep2.tile([PRE_P // 3, KT], f16, tag="ctap")
                for cc in range(3):
                    sl = slice(cc * CPQ, (cc + 1) * CPQ)
                    d2_ps = psp.tile([G * NQ, CPQ], f32, tag="d2")
                    nc.tensor.matmul(d2_ps[:], rmask_t[:], sq_t[:, sl],
                                     start=True, stop=True)
                    nc.scalar.activation(ctap_t[:, sl], d2_ps[:], AF.Exp,
                                         scale=-50.0,
                                         bias=lns_t[0:24, ki:ki + 1])
                if ki in DMA_TAPS:
                    nc.sync.dma_start(
                        wbounce_d.ap()[:, ki * KT:(ki + 1) * KT], ctap_t[:])
                else:
                    # PE replication [(qt,g) -> (g,c)] with K=24 masked
                    # stationaries (col-block qt active for rows (qt,*)).
                    for qt in range(NQ):
                        st = st4_t[:, qt * P:(qt + 1) * P]
                        for cc, (o0, o1) in enumerate(
                                ((0, 512), (512, 1024), (1024, KT))):
                            b_ps = bcp.tile([P, 512], f32, tag="bc",
                                            name="b_ps")
                            nc.tensor.matmul(b_ps[:, 0:o1 - o0], st,
                                             ctap_t[:, o0:o1],
                                             start=True, stop=True)
                            wdst = w_all[:, (qt * 12 + ki) * KT + o0:
                                         (qt * 12 + ki) * KT + o1]
                            if (qt + cc) % 2 == 0:
                                nc.scalar.copy(wdst, b_ps[:, 0:o1 - o0])
                            else:
                                nc.vector.tensor_copy(wdst,
                                                      b_ps[:, 0:o1 - o0])

            for ki in list(DMA_TAPS) + list(COMP_TAPS):
                do_tap(ki)
            # DMA-half broadcast via DRAM bounce, grouped (ch, tap-triple)
            # so each start is [6 parts, 4 qt-blocks of 7776 B].
            for ch in range(C):
                for k0 in (6, 9):
                    src_ap = wbounce_d.ap().rearrange(
                        "(q g) (k f) -> g q k f", g=G, f=KT)[
                        :, :, k0:k0 + 3, :]
                    wall_g = w_all[:].rearrange(
                        "(g c) (q k f) -> g c q k f", c=C, k=12, f=KT)
                    nc.sync.dma_start(
                        wall_g[:, ch, :, k0:k0 + 3, :], src_ap)

            # init: q0 = softmax(logits), rotating PSUM banks (z pool is
            # not allocated yet -- front pools own the banks).
            lg3i = lg_t[:].rearrange("p (r x) -> p r x", x=W)
            for c in range(NCH):
                iz = izp.tile([P, CH], f32, tag="iz", name="iz")
                nc.tensor.matmul(iz[:], iden_t[:],
                                 lg2_t[:, c * CH:(c + 1) * CH],
                                 start=True, stop=False,
                                 skip_group_check=True)
                e_t = e_pool.tile([P, CH], f16, tag="E")
                nc.scalar.activation(e_t[:], iz[:], AF.Exp)
                d_ps = d_pool.tile([G, CH], f32, tag="D")
                nc.tensor.matmul(d_ps[:], onesd_t[:], e_t[:],
                                 start=True, stop=True)
                ln_t = ln_pool.tile([G, CH], f16, tag="ln")
                nc.scalar.activation(ln_t[:], d_ps[:], AF.Ln)
                nc.tensor.matmul(iz[:], bneg_t[:], ln_t[:],
                                 start=False, stop=True,
                                 skip_group_check=True)
                nc.scalar.activation(q3a[:, 2 + 2 * c:4 + 2 * c, 2:2 + W],
                                     iz[:], AF.Exp)
                if c == 0:
                    nc.sync.dma_start(q3a[0:105, 16:18, 0:NV],
                                      q3a[21:126, 2:4, 0:NV])
                elif c == 6:
                    nc.sync.dma_start(q3a[21:126, 0:2, 0:NV],
                                      q3a[0:105, 14:16, 0:NV])

        # ---------------- iteration machinery ----------------
        z_pool = ctx.enter_context(tc.tile_pool(name="zps", bufs=1,
                                                space="PSUM"))
        t_pool = ctx.enter_context(tc.tile_pool(name="tprod", bufs=3))

        def z_banks():
            return [z_pool.tile([P, CH], f32, tag=f"z{c}", name=f"z{c}")
                    for c in range(NCH)]

        lg3 = lg_t[:].rearrange("p (r x) -> p r x", x=W)

        def tail(zs, chunks, q3n, last):
            """exp -> D-reduce -> ln -> -lnD broadcast -> exp(q).
            Halo-refresh DMAs for the just-written q fire right after the
            producing chunk (0 -> up-halo, 6 -> down-halo)."""
            for c in chunks:
                e_t = e_pool.tile([P, CH], f16, tag="E")
                nc.scalar.activation(e_t[:], zs[c][:], AF.Exp)
                d_ps = d_pool.tile([G, CH], f32, tag="D")
                nc.tensor.matmul(d_ps[:], onesd_t[:], e_t[:],
                                 start=True, stop=True)
                ln_t = ln_pool.tile([G, CH], f16, tag="ln")
                nc.scalar.activation(ln_t[:], d_ps[:], AF.Ln)
                nc.tensor.matmul(zs[c][:], bneg_t[:], ln_t[:],
                                 start=False, stop=True,
                                 skip_group_check=True)
                if last:
                    nc.scalar.activation(lg3[:, 2 * c:2 * c + 2, 0:W],
                                         zs[c][:], AF.Exp)
                    r2 = slice(2 * c * W, (2 * c + 2) * W)
                    nc.sync.dma_start(qout_d.ap()[:, r2], lg_t[:, r2])
                else:
                    nc.scalar.activation(
                        q3n[:, 2 + 2 * c:4 + 2 * c, 2:2 + W], zs[c][:],
                        AF.Exp)
                    if c == 0:
                        nc.sync.dma_start(q3n[0:105, 16:18, 0:NV],
                                          q3n[21:126, 2:4, 0:NV])
                    elif c == 6:
                        nc.sync.dma_start(q3n[21:126, 0:2, 0:NV],
                                          q3n[0:105, 14:16, 0:NV])

        # two row-halves per iteration: tails of half A hide under half B's
        # DVE/PE work; ping-pong q (read old, write new) makes that legal.
        HALVES = [(0, 8, (0, 1, 2, 3)), (8, RG, (6, 4, 5))]

        for it in range(NUM_ITERS):
            last = it == NUM_ITERS - 1
            q3o, q3n = (q3a, q3b) if it % 2 == 0 else (q3b, q3a)
            zs = z_banks()
            for (r0, r1, chunks) in HALVES:
                nr = r1 - r0
                for c in chunks:
                    sl = slice(c * CH, (c + 1) * CH)
                    nc.tensor.matmul(zs[c][:], iden_t[:], lg2_t[:, sl],
                                     start=True, stop=False,
                                     skip_group_check=True)
                    nc.tensor.matmul(zs[c][:], mneg2_t[:],
                                     q3o[:, 2 + 2 * c:4 + 2 * c, 2:2 + W],
                                     start=False, stop=False,
                                     skip_group_check=True)
                for ki, (dy, dx) in enumerate(TAPS):
                    w4 = w_all[:].rearrange(
                        "p (q k t v) -> p q k t v", q=NQ, k=12,
                        v=SS)[:, :, ki]
                    for (qdy, qdx, wdy, wdx) in ((dy, dx, 0, 0),
                                                 (-dy, -dx, -dy, -dx)):
                        q_ap = q3o[:, 2 + qdy + r0:2 + qdy + r1,
                                   2 + qdx:2 + qdx + W].rearrange(
                            "p r (q x) -> p r q x", x=XW)
                        w_ap = w4[:, :, 2 + wdy + r0:2 + wdy + r1,
                                  2 + wdx:2 + wdx + XW].rearrange(
                            "p q r x -> p r q x")
                        t_t = t_pool.tile([P, nr * W], f16, tag=f"t{r0}",
                                          name="t_t")
                        t4 = t_t[:].rearrange("p (r q x) -> p r q x",
                                              q=NQ, x=XW)
                        nc.vector.tensor_mul(t4[:, :, :, :], q_ap, w_ap)
                        for c in chunks:
                            sl = slice((2 * c - r0) * W, (2 * c + 2 - r0) * W)
                            nc.tensor.matmul(
                                zs[c][:], mneg_t[:], t_t[:, sl],
                                start=False, stop=False,
                                skip_group_check=True)
                tail(zs, chunks, q3n, last=last)

    _legalize_matmul_waits(nc, mybir)
    return nc


def _legalize_matmul_waits(nc, mybir, max_waits=2):
    """TRN2 ISA sync-wait structs hold few waits per instruction; codegen
    aborts on more. Move excess waits onto InstNoOps (1 wait each) inserted
    right before on the same engine."""
    cap = {}
    for f in nc.m.functions:
        for blk in f.blocks:
            insts = blk.instructions
            out = []
            changed = False
            for i in insts:
                si = getattr(i, "sync_info", None)
                eng = getattr(i, "engine", None)
                max_waits = cap.get(type(i).__name__, 1)
                if (si is not None and eng is not None
                        and len(si.on_wait) > max_waits):
                    waits = list(si.on_wait)
                    keep, move = [], []
                    for w in waits:
                        if "PE" in w.ant_name and len(keep) < max_waits:
                            keep.append(w)
                        else:
                            move.append(w)
                    while len(keep) < max_waits and move:
                        keep.append(move.pop())
                    nop_cap = cap.get("InstNoOp", 1)
                    while move:
                        grp, move = move[:nop_cap], move[nop_cap:]
                        nop = mybir.InstNoOp(
                            name=nc.get_next_instruction_name(),
                            engine=eng, ins=[], outs=[])
                        nop.sync_info = mybir.SyncInfo(on_wait=grp,
                                                       on_update=[])
                        out.append(nop)
                    i.sync_info = mybir.SyncInfo(
                        on_wait=keep, on_update=list(si.on_update))
                    changed = True
                out.append(i)
            if changed:
                blk.instructions = out


def _prep_shards(logits, img, compat):
    """Host-side shard prep -> list of 8 in_maps."""
    mneg = np.kron(np.eye(G), -compat.T.astype(np.float64)).astype(np.float16)
    mneg2 = ((1.0 + WC) * np.kron(np.eye(G), -compat.T.astype(np.float64))
             ).astype(np.float16)
    iden = np.eye(P, dtype=np.float16)
    st4 = np.zeros((G * NQ, NQ * P), np.float16)
    for qt in range(NQ):
        for g in range(G):
            st4[qt * G + g, qt * P + g * C:qt * P + (g + 1) * C] = 1.0
    onesd = np.kron(np.eye(G), np.ones((C, 1))).astype(np.float16)
    bneg = np.kron(np.eye(G), -np.ones((1, C))).astype(np.float16)
    # rmask [(g,rgb,q), (g,q)]: sums rgb
    rmask = np.zeros((PRE_P, G * NQ), np.float16)
    for g in range(G):
        for rgb in range(3):
            for qt in range(NQ):
                rmask[(g * 3 + rgb) * NQ + qt, qt * G + g] = 1.0
    lns = np.tile(
        np.array([math.log(SW[2 + dy, 2 + dx]) for (dy, dx) in TAPS],
                 np.float32)[None, :], (G * NQ, 1))

    in_maps = []
    for core in range(8):
        b, j = divmod(core, 4)
        s = STARTS[j]
        lg = logits[b, :, s:s + 84, :].reshape(C, G, RG, W)
        lg = np.ascontiguousarray(
            lg.transpose(1, 0, 2, 3).reshape(P, F)).astype(np.float32)
        im = np.zeros((G, 3, NQ, IM_U, IM_V), np.float32)
        for g in range(G):
            rbase = s + g * RG - 4
            u0, u1 = max(0, -rbase), min(IM_U, H - rbase)
            for qt in range(NQ):
                cbase = qt * XW - 4
                v0, v1 = max(0, -cbase), min(IM_V, W - cbase)
                im[g, :, qt, u0:u1, v0:v1] = img[
                    b, :, rbase + u0:rbase + u1, cbase + v0:cbase + v1]
        im = im.reshape(PRE_P, IM_U * IM_V).astype(np.float16)
        in_maps.append({
            "lg": lg, "img": np.ascontiguousarray(im),
            "mneg": mneg, "mneg2": mneg2, "st4": st4, "iden": iden, "onesd": onesd,
            "bneg": bneg, "rmask": rmask, "lns": lns,
        })
    return in_maps


def kernel(**inputs):
    logits = np.asarray(inputs["logits"], dtype=np.float32)
    img = np.asarray(inputs["img"], dtype=np.float32)
    compat = np.asarray(inputs["compat_mat"], dtype=np.float32)

    from concourse.bass_utils import run_bass_kernel_spmd
    if "nc" not in _BASS_CACHE:
        _BASS_CACHE["nc"] = _build_bass()
    nc = _BASS_CACHE["nc"]

    in_maps = _prep_shards(logits, img, compat)
    res = run_bass_kernel_spmd(nc, in_maps, core_ids=list(range(8)))
    _BASS_CACHE["last_result"] = res

    out = np.zeros((B, C, H, W), np.float32)
    for core in range(8):
        b, j = divmod(core, 4)
        s = STARTS[j]
        lo, hi = OWN[j]
        qc = res.results[core]["qout"].reshape(G, C, RG, W)
        qc = qc.transpose(1, 0, 2, 3).reshape(C, 84, W)
        out[b, :, s + lo:s + hi, :] = qc[:, lo:hi, :]
    return out
